# revision 7
# baseline (speedup 1.0000x reference)
"""Self-contained Trainium2 Bass kernel for the 2-layer GAT
(nn_GAT_18915035971953): 100000 nodes, 1.6M edges, 8 NeuronCores.

Strategy: edges sorted by destination and dst-sharded across 8 cores
(12500 dst nodes each). Per 128-dst window, per-edge source rows are
fetched with dma_gather; gathers are batched over GSW=4 windows per
src-bucket so each SWDGE call carries ~2k indices (994ns fixed
desc-gen cost amortized). Edge softmax weights are computed on-chip;
per-edge a_d comes from a per-slot transpose of the one-hot dst matrix
(PT path) for BOTH layers -- no second gather. The segment sum is a
one-hot matmul into PSUM. Layer 2 gathers precomputed per-head
h2 = relu(out1+b1) @ W2 rows (bf16, 512B) so no post-aggregation W2
matmul is needed. Three SPMD launches: dense layer-1 tables ->
layer-1 edge phase -> layer-2 edge phase; the host only reorders
indices and concatenates shard outputs between launches.
"""
import sys
from dataclasses import dataclass
import numpy as np
import ml_dtypes

if "/opt/trn_rl_repo" not in sys.path:
    sys.path.insert(0, "/opt/trn_rl_repo")

import concourse.bacc as bacc
import concourse.mybir as mybir
import concourse.tile as tile
from concourse.masks import make_identity
from concourse import bass_utils

# ---------------- constants ----------------

P = 128
H = 8
NQ = 4            # SWDGE queues
GSW = 4           # windows per gather supergroup
MAXNIDX = 1024    # max idxs per dma_gather call (HW limit)
F32 = mybir.dt.float32
BF16 = mybir.dt.bfloat16
I16 = mybir.dt.int16
AF = mybir.ActivationFunctionType
ALU = mybir.AluOpType
AX = mybir.AxisListType


@dataclass
class Dims:
    N: int = 100000
    NCORES: int = 8
    NBUCK: int = 4           # src buckets (int16 gather indices < 32768)

    @property
    def NPC(self):
        return self.N // self.NCORES

    @property
    def BUCK(self):
        return self.N // self.NBUCK

    @property
    def NWIN(self):
        return (self.NPC + P - 1) // P

    @property
    def NPAD(self):
        return self.NWIN * P


# ---------------- host-side index preprocessing ----------------


def _wrap16(idx):
    n = idx.shape[0]
    assert n % 16 == 0
    w = idx.reshape(n // 16, 16).T.astype(np.int16)
    return np.tile(w, (8, 1))


def build_plans(edge_index, dims: Dims):
    """Slot layout: for each supergroup sw (GSW windows), for each src
    bucket b, the (w, b) segments of sw's windows are concatenated (each
    padded to a 128 multiple) and fetched with ONE dma_gather call.

    Returns (plan, streams).
    plan: dict(supers=[{S, slot0, gcol0?, calls=[(b, n, gcol0, scol)]},...],
               windows=[{w, sw, segs=[(scol, ns)], r}], slots, gcols)
      scol is slot offset LOCAL to the supergroup.
    streams (per core): gidx int16 [128, gcols], dstloc bf16 [128, slots].
    Pad slots: gather row 0 of the bucket (finite data), dstloc = -1
    (one-hot row all-zero -> edge excluded from aggregation).
    """
    N, NC, NB, BUCK = dims.N, dims.NCORES, dims.NBUCK, dims.BUCK
    NPC, NWIN = dims.NPC, dims.NWIN
    src = np.asarray(edge_index[0], np.int64)
    dst = np.asarray(edge_index[1], np.int64)
    order = np.argsort(dst, kind="stable")
    s_src, s_dst = src[order], dst[order]
    counts = np.bincount(s_dst, minlength=N)
    node_start = np.concatenate([[0], np.cumsum(counts)])

    seg = [[[None] * NB for _ in range(NWIN)] for _ in range(NC)]
    for c in range(NC):
        d0 = c * NPC
        for w in range(NWIN):
            lo = node_start[d0 + w * P]
            hi = node_start[min(d0 + (w + 1) * P, d0 + NPC)]
            esrc = s_src[lo:hi]
            edst = s_dst[lo:hi]
            for b in range(NB):
                m = (esrc // BUCK) == b
                seg[c][w][b] = (esrc[m] - b * BUCK, edst[m] - (d0 + w * P))

    # uniform (max-over-cores) padded slot counts per (w, b)
    nn = np.zeros((NWIN, NB), np.int64)
    for w in range(NWIN):
        for b in range(NB):
            kmax = max(seg[c][w][b][0].shape[0] for c in range(NC))
            nn[w, b] = ((kmax + P - 1) // P) * P
    # ensure every window has >= 1 slot (PSUM agg needs one matmul)
    for w in range(NWIN):
        if nn[w].sum() == 0:
            nn[w, 0] = P

    supers = []
    windows = [dict(w=w, sw=w // GSW, segs=[],
                    r=min(P, NPC - w * P)) for w in range(NWIN)]
    core_g = [[] for _ in range(NC)]
    core_dl = [[] for _ in range(NC)]
    gcol0 = 0
    slot0 = 0
    for sw0 in range(0, NWIN, GSW):
        ws = list(range(sw0, min(sw0 + GSW, NWIN)))
        calls = []
        scol = 0
        for b in range(NB):
            nb_tot = int(sum(nn[w, b] for w in ws))
            if nb_tot == 0:
                continue
            for w in ws:
                k = int(nn[w, b])
                if k == 0:
                    continue
                windows[w]['segs'].append((scol, k // P))
                for c in range(NC):
                    es, ed = seg[c][w][b]
                    kk = es.shape[0]
                    gi = np.concatenate([es, np.zeros(k - kk, np.int64)])
                    dl = np.concatenate([ed, np.full(k - kk, -1, np.int64)])
                    core_g[c].append(_wrap16(gi))
                    core_dl[c].append(dl.reshape(k // P, P).T)
                scol += k // P
            # split call by MAXNIDX
            off = 0
            while off < nb_tot:
                take = min(MAXNIDX, nb_tot - off)
                calls.append((b, take, gcol0 + off // 16,
                              (scol - nb_tot // P) + off // P))
                off += take
            gcol0 += nb_tot // 16
        S = scol
        supers.append(dict(S=S, slot0=slot0, calls=calls, windows=ws))
        slot0 += S

    plan = dict(supers=supers, windows=windows, slots=slot0, gcols=gcol0)
    streams = []
    for c in range(NC):
        streams.append(dict(
            gidx=np.ascontiguousarray(np.concatenate(core_g[c], axis=1)),
            dstloc=np.ascontiguousarray(
                np.concatenate(core_dl[c], axis=1).astype(ml_dtypes.bfloat16)),
        ))
    return plan, streams


# ---------------- kernel builders ----------------


def build_dense1(dims: Dims):
    """h1 = x @ W1 (bf16 matmul), a_s1/a_d1 folds. Outputs:
    T1 [NPAD, 384] bf16 (0:256 feats, 256:272 a_s as bitcast f32),
    AD1 [NPAD, 16] f32 (0:8 a_d)."""
    NPC, NPAD = dims.NPC, dims.NPAD
    nc = bacc.Bacc(None, target_bir_lowering=False, num_swdge_queues=NQ)
    with tile.TileContext(nc) as tc:
        with tc.tile_pool(name="dram", bufs=1, space="DRAM") as dram:
            xT = dram.tile([P, NPC], BF16, kind="ExternalInput")
            W1 = dram.tile([P, 256], F32, kind="ExternalInput")
            att1 = dram.tile([1, 512], F32, kind="ExternalInput")
            T1 = dram.tile([NPAD, 384], BF16, kind="ExternalOutput")
            AD1 = dram.tile([NPAD, 16], F32, kind="ExternalOutput")
            names = dict(xT=xT.name, W1=W1.name, att1=att1.name,
                         T1=T1.name, AD1=AD1.name)
            with tc.tile_pool(name="cst", bufs=1) as cst, \
                 tc.tile_pool(name="wk", bufs=3) as wk, \
                 tc.tile_pool(name="ps", bufs=4, space="PSUM") as ps:
                xTs = cst.tile([P, NPC], BF16)
                nc.sync.dma_start(xTs[:], xT[:])
                W1s = cst.tile([P, 256], F32)
                nc.sync.dma_start(W1s[:], W1[:])
                att_s = cst.tile([1, 512], F32)
                nc.sync.dma_start(att_s[:], att1[:])
                attb = cst.tile([P, 512], F32)
                nc.gpsimd.partition_broadcast(attb[:, 0:256], att_s[0:1, 0:256])
                nc.gpsimd.partition_broadcast(attb[:, 256:512], att_s[0:1, 256:512])
                tmp = cst.tile([P, 512], F32)
                nc.vector.tensor_tensor(out=tmp[:, 0:256], in0=W1s[:],
                                        in1=attb[:, 0:256], op=ALU.mult)
                nc.vector.tensor_tensor(out=tmp[:, 256:512], in0=W1s[:],
                                        in1=attb[:, 256:512], op=ALU.mult)
                tv = tmp[:].rearrange("p (v h f) -> p v h f", v=2, h=H)
                folds = cst.tile([P, 16], F32)
                nc.vector.tensor_reduce(out=folds[:, 0:8], in_=tv[:, 0],
                                        axis=AX.X, op=ALU.add)
                nc.vector.tensor_reduce(out=folds[:, 8:16], in_=tv[:, 1],
                                        axis=AX.X, op=ALU.add)
                rhs = cst.tile([P, 272], BF16)
                nc.vector.tensor_copy(rhs[:, 0:256], W1s[:])
                nc.vector.tensor_copy(rhs[:, 256:272], folds[:])
                ntile = (NPC + P - 1) // P
                for i in range(ntile):
                    r = min(P, NPC - i * P)
                    po = ps.tile([P, 272], F32, tag="po")
                    nc.tensor.matmul(out=po[:r, :], lhsT=xTs[:, i * P:i * P + r],
                                     rhs=rhs[:], start=True, stop=True)
                    t1t = wk.tile([P, 272], BF16, tag="t1t")
                    nc.scalar.copy(t1t[:r, 0:256], po[:r, 0:256])
                    nc.vector.tensor_copy(
                        t1t[:, 256:272].bitcast(F32)[:r, 0:8],
                        po[:r, 256:264])
                    nc.sync.dma_start(T1[i * P:i * P + r, 0:272], t1t[:r, :])
                    adt = wk.tile([P, 8], F32, tag="adt")
                    nc.vector.tensor_copy(adt[:r, :], po[:r, 264:272])
                    nc.sync.dma_start(AD1[i * P:i * P + r, 0:8], adt[:r, :])
    nc.compile()
    return nc, names


def build_edge(layer, plan, dims: Dims):
    """Edge phase for layer 1 or 2.

    layer 1: gathers T1 rows [384 bf16] (h1 256 + a_s bitcast f32),
      aggregates per-head h1*alpha (264-col one-hot matmul), outputs
      T2 [NPAD, 256] bf16 (0:128 h2 = relu(out1)@W2 per head,
      128:136 a_s2 bf16) and AD2 [NPAD, 16] f32.
    layer 2: gathers T2 rows [256 bf16], aggregates per-head h2*alpha
      (136-col matmul), outputs OUT [NPC, 16] f32.

    Per-edge a_d comes from per-slot PE transpose of the one-hot
    (PT path) in both layers."""
    N, NPC, BUCK, NWIN, NPAD = dims.N, dims.NPC, dims.BUCK, dims.NWIN, dims.NPAD
    supers, windows = plan['supers'], plan['windows']
    gcols, slots = plan['gcols'], plan['slots']
    GW = 384 if layer == 1 else 256
    FW = 256 if layer == 1 else 128   # feature width in gathered row
    AGG = FW + 8
    nc = bacc.Bacc(None, target_bir_lowering=False, num_swdge_queues=NQ)
    qctr = [0]

    def nextq():
        q = qctr[0] % NQ
        qctr[0] += 1
        return q

    with tile.TileContext(nc) as tc:
        with tc.tile_pool(name="dram", bufs=1, space="DRAM") as dram:
            names = {}
            Gt = dram.tile([N, GW], BF16, kind="ExternalInput")
            ADt = dram.tile([NPAD, 16], F32, kind="ExternalInput")
            nb = 32 if layer == 1 else 16
            bias = dram.tile([1, nb], F32, kind="ExternalInput")
            gidx = dram.tile([P, gcols], I16, kind="ExternalInput")
            dstloc = dram.tile([P, slots], BF16, kind="ExternalInput")
            names.update(G=Gt.name, AD=ADt.name, bias=bias.name,
                         gidx=gidx.name, dstloc=dstloc.name)
            if layer == 1:
                W2 = dram.tile([32, 128], F32, kind="ExternalInput")
                att2 = dram.tile([1, 256], F32, kind="ExternalInput")
                T2o = dram.tile([NPAD, 256], BF16, kind="ExternalOutput")
                AD2o = dram.tile([NPAD, 16], F32, kind="ExternalOutput")
                names.update(W2=W2.name, att2=att2.name, T2=T2o.name,
                             AD2=AD2o.name)
            else:
                OUT = dram.tile([NPC, 16], F32, kind="ExternalOutput")
                names.update(OUT=OUT.name)

            with tc.tile_pool(name="cst", bufs=1) as cst, \
                 tc.tile_pool(name="gp", bufs=2) as gp, \
                 tc.tile_pool(name="gi", bufs=2) as gip, \
                 tc.tile_pool(name="wk", bufs=2) as wk, \
                 tc.tile_pool(name="psa", bufs=2, space="PSUM") as psa, \
                 tc.tile_pool(name="pst", bufs=2, space="PSUM") as pst, \
                 tc.tile_pool(name="pso", bufs=1, space="PSUM") as pso, \
                 tc.tile_pool(name="psd", bufs=2, space="PSUM") as psd:
                dstloc_s = cst.tile([P, slots], BF16)
                nc.sync.dma_start(dstloc_s[:], dstloc[:])
                ad_s = cst.tile([P, NWIN, 16], F32)
                nc.sync.dma_start(
                    ad_s[:], ADt[:].rearrange("(w p) c -> p w c", p=P))
                ad_v = ad_s[:]
                iota_i = cst.tile([P, P], mybir.dt.int32)
                nc.gpsimd.iota(iota_i[:], pattern=[[1, P]], base=0,
                               channel_multiplier=0)
                iota_bf = cst.tile([P, P], BF16)
                nc.vector.tensor_copy(iota_bf[:], iota_i[:])
                bias_s = cst.tile([1, nb], F32)
                nc.sync.dma_start(bias_s[:], bias[:])
                bias_b = cst.tile([P, nb], F32)
                nc.gpsimd.partition_broadcast(bias_b[:], bias_s[0:1, :])
                ident_bf = cst.tile([P, P], BF16)
                make_identity(nc, ident_bf[:])
                if layer == 1:
                    W2s = cst.tile([32, 128], F32)
                    nc.sync.dma_start(W2s[:], W2[:])
                    att2_s = cst.tile([1, 256], F32)
                    nc.sync.dma_start(att2_s[:], att2[:])
                    att2b = cst.tile([32, 256], F32)
                    nc.gpsimd.partition_broadcast(att2b[:, 0:128],
                                                  att2_s[0:1, 0:128])
                    nc.gpsimd.partition_broadcast(att2b[:, 128:256],
                                                  att2_s[0:1, 128:256])
                    tmp2 = cst.tile([32, 256], F32)
                    nc.vector.tensor_tensor(out=tmp2[:, 0:128], in0=W2s[:],
                                            in1=att2b[:, 0:128], op=ALU.mult)
                    nc.vector.tensor_tensor(out=tmp2[:, 128:256], in0=W2s[:],
                                            in1=att2b[:, 128:256], op=ALU.mult)
                    t2v = tmp2[:].rearrange("p (v h f) -> p v h f", v=2, h=H)
                    # W2cat = [W2 | a_s2 fold | a_d2 fold]  [32, 144]
                    W2cat = cst.tile([32, 144], F32)
                    nc.vector.tensor_copy(W2cat[:, 0:128], W2s[:])
                    nc.vector.tensor_reduce(out=W2cat[:, 128:136],
                                            in_=t2v[:, 0], axis=AX.X,
                                            op=ALU.add)
                    nc.vector.tensor_reduce(out=W2cat[:, 136:144],
                                            in_=t2v[:, 1], axis=AX.X,
                                            op=ALU.add)
                    ident = cst.tile([P, P], F32)
                    make_identity(nc, ident[:])

                for sup in supers:
                    S = sup['S']
                    ncols = sum(n for (_, n, _, _) in sup['calls']) // 16
                    gc_base = sup['calls'][0][2]
                    gidx_t = gip.tile([P, ncols], I16, tag="gi")
                    nc.sync.dma_start(gidx_t[:],
                                      gidx[:, gc_base:gc_base + ncols])
                    g_t = gp.tile([P, S, GW], BF16, tag="g")
                    for (b, n, gc0, scol) in sup['calls']:
                        nc.gpsimd.dma_gather(
                            g_t[:, scol:scol + n // P, :],
                            Gt[b * BUCK:(b + 1) * BUCK, :],
                            gidx_t[:, gc0 - gc_base:gc0 - gc_base + n // 16],
                            n, n, GW, queue_num=nextq())
                    for w in sup['windows']:
                        win = windows[w]
                        r = win['r']
                        segs = win['segs']
                        nslot = sum(ns for (_, ns) in segs)
                        adw_b = wk.tile([P, 8], BF16, tag="adwb")
                        nc.vector.memset(adw_b[:], 0.0)
                        nc.vector.tensor_copy(adw_b[:r, :], ad_v[:r, w, 0:8])
                        agg = psa.tile([P, AGG], F32, tag="agg")
                        sdone = 0
                        for (scol, ns) in segs:
                            p_t = wk.tile([P, ns, P], BF16, tag="pt")
                            dl_b = dstloc_s[:, sup['slot0'] + scol:
                                            sup['slot0'] + scol + ns] \
                                .unsqueeze(2).to_broadcast([P, ns, P])
                            io_b = iota_bf[:].unsqueeze(1) \
                                .to_broadcast([P, ns, P])
                            nc.vector.tensor_tensor(out=p_t[:], in0=dl_b,
                                                    in1=io_b, op=ALU.is_equal)
                            adE = psd.tile([P, ns * 8], F32, tag="adE")
                            for s in range(ns):
                                ptp = pst.tile([P, P], BF16, tag="tps")
                                nc.tensor.transpose(ptp[:], p_t[:, s, :],
                                                    ident_bf[:])
                                pts = wk.tile([P, P], BF16, tag="pts")
                                nc.scalar.copy(pts[:], ptp[:])
                                nc.tensor.matmul(
                                    out=adE[:, s * 8:(s + 1) * 8],
                                    lhsT=pts[:], rhs=adw_b[:],
                                    start=True, stop=True)
                            if layer == 1:
                                a_s_ap = g_t[:, scol:scol + ns, :] \
                                    .rearrange("p s e -> p (s e)") \
                                    .bitcast(F32) \
                                    .rearrange("p (s e) -> p s e", e=192) \
                                    [:, :, 128:136]
                            else:
                                a_s_ap = g_t[:, scol:scol + ns, 128:136]
                            et = wk.tile([P, ns, 8], F32, tag="et")
                            nc.vector.tensor_tensor(
                                out=et[:], in0=a_s_ap,
                                in1=adE[:].rearrange("p (s e) -> p s e", e=8),
                                op=ALU.add)
                            nc.vector.scalar_tensor_tensor(
                                out=et[:], in0=et[:], scalar=0.2, in1=et[:],
                                op0=ALU.mult, op1=ALU.max)
                            rhs_t = wk.tile([P, ns, AGG], BF16, tag="rhs")
                            nc.scalar.activation(rhs_t[:, :, FW:FW + 8],
                                                 et[:], AF.Exp)
                            wexp_b = rhs_t[:, :, FW:FW + 8].unsqueeze(3) \
                                .to_broadcast([P, ns, 8, FW // 8])
                            g_v = g_t[:, scol:scol + ns, 0:FW] \
                                .rearrange("p s (h f) -> p s h f", h=H)
                            nc.vector.tensor_tensor(
                                out=rhs_t[:, :, 0:FW]
                                .rearrange("p s (h f) -> p s h f", h=H),
                                in0=g_v, in1=wexp_b, op=ALU.mult)
                            for s in range(ns):
                                nc.tensor.matmul(
                                    out=agg[:], lhsT=p_t[:, s, :],
                                    rhs=rhs_t[:, s, :],
                                    start=(sdone + s == 0),
                                    stop=(sdone + s == nslot - 1))
                            sdone += ns
                        zr = wk.tile([P, 8], F32, tag="zr")
                        nc.vector.tensor_scalar_add(zr[:], agg[:, FW:FW + 8],
                                                    1e-16)
                        nc.vector.reciprocal(zr[:], zr[:])
                        zrb = zr[:].unsqueeze(2).to_broadcast([P, H, FW // 8])
                        hn = wk.tile([P, FW], F32, tag="hn")
                        nc.vector.tensor_tensor(
                            out=hn[:].rearrange("p (h f) -> p h f", h=H),
                            in0=agg[:, 0:FW].rearrange("p (h f) -> p h f", h=H),
                            in1=zrb, op=ALU.mult)
                        if layer == 1:
                            o1 = wk.tile([P, 32], F32, tag="o1")
                            nc.vector.tensor_reduce(
                                out=o1[:],
                                in_=hn[:].rearrange("p (h f) -> p f h", h=H),
                                axis=AX.X, op=ALU.add)
                            nc.scalar.mul(o1[:], o1[:], 1.0 / H)
                            nc.vector.tensor_tensor(out=o1[:], in0=o1[:],
                                                    in1=bias_b[:, 0:32],
                                                    op=ALU.add)
                            nc.vector.tensor_scalar_max(o1[:], o1[:], 0.0)
                            hT = pso.tile([32, P], F32, tag="hT")
                            nc.tensor.transpose(hT[:], o1[:], ident[:])
                            hTs = wk.tile([32, P], F32, tag="hTs")
                            nc.vector.tensor_copy(hTs[:], hT[:])
                            h2a = pso.tile([P, 144], F32, tag="h2a")
                            nc.tensor.matmul(out=h2a[:], lhsT=hTs[:],
                                             rhs=W2cat[:], start=True,
                                             stop=True)
                            t2t = wk.tile([P, 136], BF16, tag="t2t")
                            nc.scalar.copy(t2t[:, 0:128], h2a[:, 0:128])
                            nc.vector.tensor_copy(t2t[:, 128:136],
                                                  h2a[:, 128:136])
                            nc.sync.dma_start(T2o[w * P:w * P + r, 0:136],
                                              t2t[:r, :])
                            ad2t = wk.tile([P, 8], F32, tag="ad2t")
                            nc.vector.tensor_copy(ad2t[:], h2a[:, 136:144])
                            nc.sync.dma_start(AD2o[w * P:w * P + r, 0:8],
                                              ad2t[:r, :])
                        else:
                            ot = wk.tile([P, 16], F32, tag="ot")
                            nc.vector.tensor_reduce(
                                out=ot[:],
                                in_=hn[:].rearrange("p (h f) -> p f h", h=H),
                                axis=AX.X, op=ALU.add)
                            nc.scalar.mul(ot[:], ot[:], 1.0 / H)
                            nc.vector.tensor_tensor(out=ot[:], in0=ot[:],
                                                    in1=bias_b[:, 0:16],
                                                    op=ALU.add)
                            nc.sync.dma_start(OUT[w * P:w * P + r, :],
                                              ot[:r, :])
    nc.compile()
    return nc, names


# ---------------- driver ----------------


def _run_pipeline(inputs, dims, trace=False):
    x = np.asarray(inputs['x'], np.float32)
    ei = np.asarray(inputs['edge_index'])
    W1 = np.ascontiguousarray(np.asarray(inputs['W1'], np.float32))
    as1 = np.asarray(inputs['att_src1'], np.float32)
    ad1 = np.asarray(inputs['att_dst1'], np.float32)
    b1 = np.asarray(inputs['b1'], np.float32)
    W2 = np.ascontiguousarray(np.asarray(inputs['W2'], np.float32))
    as2 = np.asarray(inputs['att_src2'], np.float32)
    ad2 = np.asarray(inputs['att_dst2'], np.float32)
    b2 = np.asarray(inputs['b2'], np.float32)
    NC, NPC = dims.NCORES, dims.NPC

    plan, streams = build_plans(ei, dims)
    times = {}

    nc1, n1 = build_dense1(dims)
    att1 = np.ascontiguousarray(np.concatenate(
        [as1.reshape(-1), ad1.reshape(-1)]).reshape(1, -1).astype(np.float32))
    ins1 = [{n1['xT']: np.ascontiguousarray(
                x[c * NPC:(c + 1) * NPC, :].T.astype(ml_dtypes.bfloat16)),
             n1['W1']: W1, n1['att1']: att1} for c in range(NC)]
    r1 = bass_utils.run_bass_kernel_spmd(nc1, ins1, core_ids=list(range(NC)),
                                         trace=trace)
    times['dense1'] = r1.exec_time_ns
    T1full = np.concatenate([r1.results[c][n1['T1']][:NPC] for c in range(NC)])
    ad1_shards = [r1.results[c][n1['AD1']] for c in range(NC)]

    nc2, n2 = build_edge(1, plan, dims)
    att2 = np.ascontiguousarray(np.concatenate(
        [as2.reshape(-1), ad2.reshape(-1)]).reshape(1, -1).astype(np.float32))
    ins2 = [{n2['G']: T1full, n2['AD']: ad1_shards[c], n2['W2']: W2,
             n2['att2']: att2,
             n2['bias']: np.ascontiguousarray(b1.reshape(1, -1)),
             n2['gidx']: streams[c]['gidx'],
             n2['dstloc']: streams[c]['dstloc']} for c in range(NC)]
    r2 = bass_utils.run_bass_kernel_spmd(nc2, ins2, core_ids=list(range(NC)),
                                         trace=trace)
    times['edge1'] = r2.exec_time_ns
    T2full = np.concatenate([r2.results[c][n2['T2']][:NPC] for c in range(NC)])
    ad2_shards = [r2.results[c][n2['AD2']] for c in range(NC)]

    nc3, n3 = build_edge(2, plan, dims)
    ins3 = [{n3['G']: T2full, n3['AD']: ad2_shards[c],
             n3['bias']: np.ascontiguousarray(b2.reshape(1, -1)),
             n3['gidx']: streams[c]['gidx'],
             n3['dstloc']: streams[c]['dstloc']} for c in range(NC)]
    r3 = bass_utils.run_bass_kernel_spmd(nc3, ins3, core_ids=list(range(NC)),
                                         trace=trace)
    times['edge2'] = r3.exec_time_ns
    out = np.concatenate([r3.results[c][n3['OUT']] for c in range(NC)])
    return np.ascontiguousarray(out.astype(np.float32)), times


def kernel(**inputs):
    out, _ = _run_pipeline(inputs, Dims(), trace=False)
    return out


# revision 20
# speedup vs baseline: 1.2117x; 1.2117x over previous
"""Self-contained Trainium2 Bass kernel for the 2-layer GAT
(nn_GAT_18915035971953): 100000 nodes, 1.6M edges, 8 NeuronCores.

Strategy: edges sorted by destination and dst-sharded across 8 cores
(12500 dst nodes each). Per 128-dst window, per-edge source rows are
fetched with dma_gather; gathers are batched over GSW=4 windows per
src-bucket. Per-edge a_d is produced without per-slot transposes: a
1-row matmul broadcasts the transposed dst-slot stream across
partitions, one is_equal builds the transposed one-hot (fp8), and
8-wide matmuls select a_d per edge. Edge softmax weights are computed
on-chip; the segment sum is a one-hot matmul into PSUM. Layer-1 node
tables pack h1 + a_s + a_d in one row (gathered at 576B); layer 2
gathers precomputed h2 = relu(out1+b1) @ W2 rows with a_s2/a_d2
packed. Host extracts per-dst a_d tables from the row pads between
launches (index ops only). Windows are software-pipelined (build
stage w+2 while aggregating w), outputs are written once per
supergroup. Three SPMD launches: dense tables -> edge1 -> edge2."""
import sys
from dataclasses import dataclass
import numpy as np
import ml_dtypes

if "/opt/trn_rl_repo" not in sys.path:
    sys.path.insert(0, "/opt/trn_rl_repo")

import concourse.bacc as bacc
import concourse.mybir as mybir
import concourse.tile as tile
from concourse.masks import make_identity
from concourse import bass_utils

# ---------------- constants ----------------

P = 128
H = 8
NQ = 4            # SWDGE queues
GSW = 3           # windows per gather supergroup
MAXNIDX = 1024    # max idxs per dma_gather call (HW limit)
F32 = mybir.dt.float32
BF16 = mybir.dt.bfloat16
FP8 = mybir.dt.float8e4
I16 = mybir.dt.int16
AF = mybir.ActivationFunctionType
ALU = mybir.AluOpType
AX = mybir.AxisListType


@dataclass
class Dims:
    N: int = 100000
    NCORES: int = 8
    NBUCK: int = 4           # src buckets (int16 gather indices < 32768)

    @property
    def NPC(self):
        return self.N // self.NCORES

    @property
    def BUCK(self):
        return self.N // self.NBUCK

    @property
    def NWIN(self):
        return (self.NPC + P - 1) // P

    @property
    def NPAD(self):
        return self.NWIN * P


# ---------------- host-side index preprocessing ----------------


def _wrap16(idx):
    n = idx.shape[0]
    assert n % 16 == 0
    w = idx.reshape(n // 16, 16).T.astype(np.int16)
    return np.tile(w, (8, 1))


def build_plans(edge_index, dims: Dims):
    """Slot layout: for each supergroup sw (GSW windows), for each src
    bucket b, the (w, b) segments of sw's windows are concatenated (each
    padded to a 128 multiple) and fetched with ONE dma_gather call
    (split at MAXNIDX).

    Returns (plan, streams).
    plan: dict(supers=[{S, slot0, calls=[(b, n, gcol0, scol)],
                        windows=[w...]}],
               windows=[{w, segs=[(scol, ns)]}], slots, gcols)
      scol is slot offset LOCAL to the supergroup.
    streams (per core): gidx int16 [128, gcols], dstloc bf16 [128, slots],
      dstlocT bf16 [1, slots*128] (edge-order dst-slot values).
    Pad slots: gather row 0 of the bucket (finite data), dstloc = -1
    (one-hot row all-zero -> edge excluded from aggregation)."""
    N, NC, NB, BUCK = dims.N, dims.NCORES, dims.NBUCK, dims.BUCK
    NPC, NWIN = dims.NPC, dims.NWIN
    src = np.asarray(edge_index[0], np.int64)
    dst = np.asarray(edge_index[1], np.int64)
    order = np.argsort(dst, kind="stable")
    s_src, s_dst = src[order], dst[order]
    counts = np.bincount(s_dst, minlength=N)
    node_start = np.concatenate([[0], np.cumsum(counts)])

    seg = [[[None] * NB for _ in range(NWIN)] for _ in range(NC)]
    for c in range(NC):
        d0 = c * NPC
        for w in range(NWIN):
            lo = node_start[d0 + w * P]
            hi = node_start[min(d0 + (w + 1) * P, d0 + NPC)]
            esrc = s_src[lo:hi]
            edst = s_dst[lo:hi]
            for b in range(NB):
                m = (esrc // BUCK) == b
                seg[c][w][b] = (esrc[m] - b * BUCK, edst[m] - (d0 + w * P))

    nn = np.zeros((NWIN, NB), np.int64)
    for w in range(NWIN):
        for b in range(NB):
            kmax = max(seg[c][w][b][0].shape[0] for c in range(NC))
            nn[w, b] = ((kmax + P - 1) // P) * P
    for w in range(NWIN):
        if nn[w].sum() == 0:
            nn[w, 0] = P

    supers = []
    windows = [dict(w=w, segs=[]) for w in range(NWIN)]
    core_g = [[] for _ in range(NC)]
    core_dl = [[] for _ in range(NC)]
    core_dlT = [[] for _ in range(NC)]
    gcol0 = 0
    slot0 = 0
    for sw0 in range(0, NWIN, GSW):
        ws = list(range(sw0, min(sw0 + GSW, NWIN)))
        calls = []
        scol = 0
        for b in range(NB):
            nb_tot = int(sum(nn[w, b] for w in ws))
            if nb_tot == 0:
                continue
            for w in ws:
                k = int(nn[w, b])
                if k == 0:
                    continue
                windows[w]['segs'].append((scol, k // P))
                for c in range(NC):
                    es, ed = seg[c][w][b]
                    kk = es.shape[0]
                    gi = np.concatenate([es, np.zeros(k - kk, np.int64)])
                    dl = np.concatenate([ed, np.full(k - kk, -1, np.int64)])
                    core_g[c].append(_wrap16(gi))
                    core_dl[c].append(dl.reshape(k // P, P).T)
                    core_dlT[c].append(dl)
                scol += k // P
            off = 0
            while off < nb_tot:
                take = min(MAXNIDX, nb_tot - off)
                calls.append((b, take, gcol0 + off // 16,
                              (scol - nb_tot // P) + off // P))
                off += take
            gcol0 += nb_tot // 16
        supers.append(dict(S=scol, slot0=slot0, calls=calls, windows=ws))
        slot0 += scol

    plan = dict(supers=supers, windows=windows, slots=slot0, gcols=gcol0)
    streams = []
    for c in range(NC):
        streams.append(dict(
            gidx=np.ascontiguousarray(np.concatenate(core_g[c], axis=1)),
            dstloc=np.ascontiguousarray(
                np.concatenate(core_dl[c], axis=1).astype(ml_dtypes.bfloat16)),
            dstlocT=np.ascontiguousarray(
                np.concatenate(core_dlT[c]).reshape(1, -1)
                .astype(ml_dtypes.bfloat16)),
        ))
    return plan, streams


# ---------------- kernel builders ----------------


def build_dense1(dims: Dims):
    """h1 = x @ W1 (bf16 matmul) + attention folds. Output rows
    T1 [NPAD, 384] bf16 (768B rows): cols 0:256 h1 feats, 256:272 a_s
    (bitcast f32), 272:288 a_d (bitcast f32).
    Host extracts the per-dst a_d table. xT input is host-padded to
    NPAD columns (zeros past NPC)."""
    NPAD = dims.NPAD
    NWIN = dims.NWIN
    nc = bacc.Bacc(None, target_bir_lowering=False, num_swdge_queues=NQ)
    with tile.TileContext(nc) as tc:
        with tc.tile_pool(name="dram", bufs=1, space="DRAM") as dram:
            xT = dram.tile([P, NPAD], BF16, kind="ExternalInput")
            W1 = dram.tile([P, 256], F32, kind="ExternalInput")
            att1 = dram.tile([1, 512], F32, kind="ExternalInput")
            T1 = dram.tile([NPAD, 384], BF16, kind="ExternalOutput")
            names = dict(xT=xT.name, W1=W1.name, att1=att1.name, T1=T1.name)
            with tc.tile_pool(name="cst", bufs=1) as cst, \
                 tc.tile_pool(name="wk", bufs=3) as wk, \
                 tc.tile_pool(name="ps", bufs=4, space="PSUM") as ps:
                xTs = cst.tile([P, NPAD], BF16)
                nc.sync.dma_start(xTs[:], xT[:])
                W1s = cst.tile([P, 256], F32)
                nc.sync.dma_start(W1s[:], W1[:])
                att_s = cst.tile([1, 512], F32)
                nc.sync.dma_start(att_s[:], att1[:])
                attb = cst.tile([P, 512], F32)
                nc.gpsimd.partition_broadcast(attb[:, 0:256], att_s[0:1, 0:256])
                nc.gpsimd.partition_broadcast(attb[:, 256:512], att_s[0:1, 256:512])
                tmp = cst.tile([P, 512], F32)
                nc.vector.tensor_tensor(out=tmp[:, 0:256], in0=W1s[:],
                                        in1=attb[:, 0:256], op=ALU.mult)
                nc.vector.tensor_tensor(out=tmp[:, 256:512], in0=W1s[:],
                                        in1=attb[:, 256:512], op=ALU.mult)
                tv = tmp[:].rearrange("p (v h f) -> p v h f", v=2, h=H)
                folds = cst.tile([P, 16], F32)
                nc.vector.tensor_reduce(out=folds[:, 0:8], in_=tv[:, 0],
                                        axis=AX.X, op=ALU.add)
                nc.vector.tensor_reduce(out=folds[:, 8:16], in_=tv[:, 1],
                                        axis=AX.X, op=ALU.add)
                rhs = cst.tile([P, 272], BF16)
                nc.vector.tensor_copy(rhs[:, 0:256], W1s[:])
                nc.vector.tensor_copy(rhs[:, 256:272], folds[:])
                grp = 0
                while grp * GSW < NWIN:
                    ws = list(range(grp * GSW, min((grp + 1) * GSW, NWIN)))
                    nw = len(ws)
                    t14 = wk.tile([P, nw, 288], BF16, tag="t14")
                    for j, w in enumerate(ws):
                        po = ps.tile([P, 272], F32, tag="po")
                        nc.tensor.matmul(out=po[:],
                                         lhsT=xTs[:, w * P:(w + 1) * P],
                                         rhs=rhs[:], start=True, stop=True)
                        nc.scalar.copy(t14[:, j, 0:256], po[:, 0:256])
                        nc.vector.tensor_copy(
                            t14[:, j, 256:288].bitcast(F32)[:, 0:16],
                            po[:, 256:272])
                    nc.sync.dma_start(
                        T1[ws[0] * P:(ws[-1] + 1) * P, 0:288]
                        .rearrange("(w p) c -> p w c", p=P), t14[:])
                    grp += 1
    nc.compile()
    return nc, names


def build_edge(layer, plan, dims: Dims):
    """Edge phase for layer 1 or 2.

    layer 1: gathers T1 rows (576B: h1 256 bf16 + a_s/a_d bitcast f32),
      aggregates per-head h1*alpha (264-col one-hot matmul), outputs
      T2 [NPAD, 256] bf16 rows: 0:128 h2 = relu(out1+b1)@W2 per head,
      128:136 a_s2 bf16, 136:144 a_d2 bf16.
    layer 2: gathers T2 rows (512B), aggregates per-head h2*alpha
      (136-col matmul), outputs OUT [NPAD, 16] f32.

    Per-edge a_d: 1-row matmul broadcasts the transposed dst-slot
    stream; is_equal vs the partition index builds the transposed
    one-hot (fp8); 8-wide matmuls with the fp8 a_d table select a_d."""
    N, BUCK, NWIN, NPAD = dims.N, dims.BUCK, dims.NWIN, dims.NPAD
    supers, windows = plan['supers'], plan['windows']
    gcols, slots = plan['gcols'], plan['slots']
    GELEM = 384 if layer == 1 else 256   # gathered elements per row
    GSTEP = 384 if layer == 1 else 256   # table row stride (elements)
    FW = 256 if layer == 1 else 128      # feature width
    AGG = FW + 8
    OW = 144 if layer == 1 else 16       # output row width
    nc = bacc.Bacc(None, target_bir_lowering=False, num_swdge_queues=NQ)
    qctr = [0]

    def nextq():
        q = qctr[0] % NQ
        qctr[0] += 1
        return q

    with tile.TileContext(nc) as tc:
        with tc.tile_pool(name="dram", bufs=1, space="DRAM") as dram:
            names = {}
            Gt = dram.tile([N, GSTEP], BF16, kind="ExternalInput")
            ADt = dram.tile([NPAD, 8], F32, kind="ExternalInput")
            nb = 32 if layer == 1 else 16
            bias = dram.tile([1, nb], F32, kind="ExternalInput")
            gidx = dram.tile([P, gcols], I16, kind="ExternalInput")
            dstloc = dram.tile([P, slots], BF16, kind="ExternalInput")
            dstlocT = dram.tile([1, slots * P], BF16, kind="ExternalInput")
            names.update(G=Gt.name, AD=ADt.name, bias=bias.name,
                         gidx=gidx.name, dstloc=dstloc.name,
                         dstlocT=dstlocT.name)
            if layer == 1:
                W2 = dram.tile([32, 128], F32, kind="ExternalInput")
                att2 = dram.tile([1, 256], F32, kind="ExternalInput")
                T2o = dram.tile([NPAD, 256], BF16, kind="ExternalOutput")
                names.update(W2=W2.name, att2=att2.name, T2=T2o.name)
            else:
                OUT = dram.tile([NPAD, 16], F32, kind="ExternalOutput")
                names.update(OUT=OUT.name)

            with tc.tile_pool(name="cst", bufs=1) as cst, \
                 tc.tile_pool(name="gp", bufs=2) as gp, \
                 tc.tile_pool(name="gi", bufs=2) as gip, \
                 tc.tile_pool(name="gd", bufs=1) as gdp, \
                 tc.tile_pool(name="wk1", bufs=8) as wk1, \
                 tc.tile_pool(name="wk2", bufs=4) as wk2, \
                 tc.tile_pool(name="wk3", bufs=3) as wk3, \
                 tc.tile_pool(name="wo", bufs=2) as wo, \
                 tc.tile_pool(name="psa", bufs=2, space="PSUM") as psa, \
                 tc.tile_pool(name="psb", bufs=2, space="PSUM") as psb, \
                 tc.tile_pool(name="pso", bufs=1, space="PSUM") as pso, \
                 tc.tile_pool(name="psd", bufs=2, space="PSUM") as psd:
                dstloc_s = cst.tile([P, slots], BF16)
                nc.sync.dma_start(dstloc_s[:], dstloc[:])
                ad_s = cst.tile([P, NWIN, 8], F32)
                nc.sync.dma_start(
                    ad_s[:], ADt[:].rearrange("(w p) c -> p w c", p=P))
                ad8 = cst.tile([P, NWIN, 8], FP8)
                nc.vector.tensor_copy(ad8[:], ad_s[:])
                iota_i = cst.tile([P, P], mybir.dt.int32)
                nc.gpsimd.iota(iota_i[:], pattern=[[1, P]], base=0,
                               channel_multiplier=0)
                iota_bf = cst.tile([P, P], BF16)
                nc.vector.tensor_copy(iota_bf[:], iota_i[:])
                iota_pi = cst.tile([P, 1], mybir.dt.int32)
                nc.gpsimd.iota(iota_pi[:], pattern=[[0, 1]], base=0,
                               channel_multiplier=1)
                iota_pb = cst.tile([P, 1], BF16)
                nc.vector.tensor_copy(iota_pb[:], iota_pi[:])
                ones1 = cst.tile([1, P], BF16)
                nc.vector.memset(ones1[:], 1.0)
                bias_s = cst.tile([1, nb], F32)
                nc.sync.dma_start(bias_s[:], bias[:])
                bias_b = cst.tile([P, nb], F32)
                nc.gpsimd.partition_broadcast(bias_b[:], bias_s[0:1, :])
                if layer == 1:
                    W2s = cst.tile([32, 128], F32)
                    nc.sync.dma_start(W2s[:], W2[:])
                    att2_s = cst.tile([1, 256], F32)
                    nc.sync.dma_start(att2_s[:], att2[:])
                    att2b = cst.tile([32, 256], F32)
                    nc.gpsimd.partition_broadcast(att2b[:, 0:128],
                                                  att2_s[0:1, 0:128])
                    nc.gpsimd.partition_broadcast(att2b[:, 128:256],
                                                  att2_s[0:1, 128:256])
                    tmp2 = cst.tile([32, 256], F32)
                    nc.vector.tensor_tensor(out=tmp2[:, 0:128], in0=W2s[:],
                                            in1=att2b[:, 0:128], op=ALU.mult)
                    nc.vector.tensor_tensor(out=tmp2[:, 128:256], in0=W2s[:],
                                            in1=att2b[:, 128:256], op=ALU.mult)
                    t2v = tmp2[:].rearrange("p (v h f) -> p v h f", v=2, h=H)
                    W2cat = cst.tile([32, 144], F32)
                    nc.vector.tensor_copy(W2cat[:, 0:128], W2s[:])
                    nc.vector.tensor_reduce(out=W2cat[:, 128:136],
                                            in_=t2v[:, 0], axis=AX.X,
                                            op=ALU.add)
                    nc.vector.tensor_reduce(out=W2cat[:, 136:144],
                                            in_=t2v[:, 1], axis=AX.X,
                                            op=ALU.add)
                    ident = cst.tile([P, P], F32)
                    make_identity(nc, ident[:])

                for sup in supers:
                    S = sup['S']
                    ncols = sum(n for (_, n, _, _) in sup['calls']) // 16
                    gc_base = sup['calls'][0][2]
                    gidx_t = gip.tile([P, ncols], I16, tag="gi")
                    nc.sync.dma_start(gidx_t[:],
                                      gidx[:, gc_base:gc_base + ncols])
                    dlT_t = gdp.tile([1, S * P], BF16, tag="dlt")
                    nc.sync.dma_start(
                        dlT_t[:],
                        dstlocT[0:1, sup['slot0'] * P:(sup['slot0'] + S) * P])
                    g_t = gp.tile([P, S, GELEM], BF16, tag="g")
                    for (b, n, gc0, scol) in sup['calls']:
                        nc.gpsimd.dma_gather(
                            g_t[:, scol:scol + n // P, :],
                            Gt[b * BUCK:(b + 1) * BUCK, 0:GELEM],
                            gidx_t[:, gc0 - gc_base:gc0 - gc_base + n // 16],
                            n, n, GELEM, elem_step=GSTEP, queue_num=nextq())
                    ws = sup['windows']
                    nw = len(ws)
                    out_t = wo.tile([P, nw, OW], BF16 if layer == 1 else F32,
                                    tag="out")
                    built = {}

                    def loop1(w):
                        segs = windows[w]['segs']
                        tiles = []
                        for (scol, ns) in segs:
                            pts = wk2.tile([P, ns * P], FP8, tag="pts")
                            for c0 in range(0, ns, 4):
                                cn = min(4, ns - c0)
                                pb = psb.tile([P, cn * P], F32, tag="pb")
                                nc.tensor.matmul(
                                    out=pb[:], lhsT=ones1[:],
                                    rhs=dlT_t[0:1, (scol + c0) * P:
                                              (scol + c0 + cn) * P],
                                    start=True, stop=True)
                                nc.vector.tensor_tensor(
                                    out=pts[:, c0 * P:(c0 + cn) * P],
                                    in0=pb[:],
                                    in1=iota_pb[:].to_broadcast([P, cn * P]),
                                    op=ALU.is_equal)
                            adE = psd.tile([P, ns * 8], F32, tag="adE")
                            for k in range(ns):
                                nc.tensor.matmul(
                                    out=adE[:, k * 8:(k + 1) * 8],
                                    lhsT=pts[:, k * P:(k + 1) * P],
                                    rhs=ad8[:, w, :], start=True, stop=True)
                            p_t = wk1.tile([P, ns, P], BF16, tag="pt")
                            dl_b = dstloc_s[:, sup['slot0'] + scol:
                                            sup['slot0'] + scol + ns] \
                                .unsqueeze(2).to_broadcast([P, ns, P])
                            io_b = iota_bf[:].unsqueeze(1) \
                                .to_broadcast([P, ns, P])
                            nc.vector.tensor_tensor(out=p_t[:], in0=dl_b,
                                                    in1=io_b, op=ALU.is_equal)
                            if layer == 1:
                                a_s_ap = g_t[:] \
                                    .rearrange("p s e -> p (s e)") \
                                    .bitcast(F32) \
                                    .rearrange("p (s e) -> p s e", e=192) \
                                    [:, scol:scol + ns, 128:136]
                            else:
                                a_s_ap = g_t[:, scol:scol + ns, 128:136]
                            et = wk2.tile([P, ns, 8], F32, tag="et")
                            nc.vector.tensor_tensor(
                                out=et[:], in0=a_s_ap,
                                in1=adE[:].rearrange("p (s e) -> p s e", e=8),
                                op=ALU.add)
                            nc.vector.scalar_tensor_tensor(
                                out=et[:], in0=et[:], scalar=0.2, in1=et[:],
                                op0=ALU.mult, op1=ALU.max)
                            rhs_t = wk1.tile([P, ns, AGG], BF16, tag="rhs")
                            nc.scalar.activation(rhs_t[:, :, FW:FW + 8],
                                                 et[:], AF.Exp)
                            wexp_b = rhs_t[:, :, FW:FW + 8].unsqueeze(3) \
                                .to_broadcast([P, ns, 8, FW // 8])
                            g_v = g_t[:, scol:scol + ns, 0:FW] \
                                .rearrange("p s (h f) -> p s h f", h=H)
                            nc.vector.tensor_tensor(
                                out=rhs_t[:, :, 0:FW]
                                .rearrange("p s (h f) -> p s h f", h=H),
                                in0=g_v, in1=wexp_b, op=ALU.mult)
                            tiles.append((p_t, rhs_t, ns))
                        built[w] = tiles

                    def loop2(w, wi):
                        tiles = built.pop(w)
                        nslot = sum(ns for (_, _, ns) in tiles)
                        agg = psa.tile([P, AGG], F32, tag="agg")
                        sdone = 0
                        for (p_t, rhs_t, ns) in tiles:
                            for k in range(ns):
                                nc.tensor.matmul(
                                    out=agg[:], lhsT=p_t[:, k, :],
                                    rhs=rhs_t[:, k, :],
                                    start=(sdone + k == 0),
                                    stop=(sdone + k == nslot - 1))
                            sdone += ns
                        zr = wk3.tile([P, 8], F32, tag="zr")
                        nc.vector.tensor_scalar_add(zr[:], agg[:, FW:FW + 8],
                                                    1e-16)
                        nc.vector.reciprocal(zr[:], zr[:])
                        nc.vector.tensor_scalar_mul(zr[:], zr[:], 1.0 / H)
                        zrb = zr[:].unsqueeze(2).to_broadcast([P, H, FW // 8])
                        hn = wk3.tile([P, FW], F32, tag="hn")
                        nc.vector.tensor_tensor(
                            out=hn[:].rearrange("p (h f) -> p h f", h=H),
                            in0=agg[:, 0:FW].rearrange("p (h f) -> p h f", h=H),
                            in1=zrb, op=ALU.mult)
                        if layer == 1:
                            o1 = wk3.tile([P, 32], F32, tag="o1")
                            nc.vector.tensor_reduce(
                                out=o1[:],
                                in_=hn[:].rearrange("p (h f) -> p f h", h=H),
                                axis=AX.X, op=ALU.add)
                            nc.vector.tensor_tensor(out=o1[:], in0=o1[:],
                                                    in1=bias_b[:, 0:32],
                                                    op=ALU.add)
                            nc.vector.tensor_scalar_max(o1[:], o1[:], 0.0)
                            hT = pso.tile([32, P], F32, tag="hT")
                            nc.tensor.transpose(hT[:], o1[:], ident[:])
                            hTs = wk3.tile([32, P], F32, tag="hTs")
                            nc.vector.tensor_copy(hTs[:], hT[:])
                            h2a = pso.tile([P, 144], F32, tag="h2a")
                            nc.tensor.matmul(out=h2a[:], lhsT=hTs[:],
                                             rhs=W2cat[:], start=True,
                                             stop=True)
                            nc.scalar.copy(out_t[:, wi, :], h2a[:])
                        else:
                            nc.vector.tensor_reduce(
                                out=out_t[:, wi, :],
                                in_=hn[:].rearrange("p (h f) -> p f h", h=H),
                                axis=AX.X, op=ALU.add)
                            nc.vector.tensor_tensor(out=out_t[:, wi, :],
                                                    in0=out_t[:, wi, :],
                                                    in1=bias_b[:, 0:16],
                                                    op=ALU.add)

                    prev = []
                    for w in ws:
                        loop1(w)
                        prev.append(w)
                        if len(prev) > 1:
                            loop2(prev[0], ws.index(prev[0]))
                            prev.pop(0)
                    for w in prev:
                        loop2(w, ws.index(w))
                    dst_ap = (T2o if layer == 1 else OUT)
                    nc.sync.dma_start(
                        dst_ap[ws[0] * P:(ws[-1] + 1) * P, 0:OW]
                        .rearrange("(w p) c -> p w c", p=P), out_t[:])
    nc.compile()
    return nc, names


# ---------------- driver ----------------


def _run_pipeline(inputs, dims, trace=False):
    x = np.asarray(inputs['x'], np.float32)
    ei = np.asarray(inputs['edge_index'])
    W1 = np.ascontiguousarray(np.asarray(inputs['W1'], np.float32))
    as1 = np.asarray(inputs['att_src1'], np.float32)
    ad1 = np.asarray(inputs['att_dst1'], np.float32)
    b1 = np.asarray(inputs['b1'], np.float32)
    W2 = np.ascontiguousarray(np.asarray(inputs['W2'], np.float32))
    as2 = np.asarray(inputs['att_src2'], np.float32)
    ad2 = np.asarray(inputs['att_dst2'], np.float32)
    b2 = np.asarray(inputs['b2'], np.float32)
    NC, NPC, NPAD = dims.NCORES, dims.NPC, dims.NPAD

    plan, streams = build_plans(ei, dims)
    times = {}

    nc1, n1 = build_dense1(dims)
    att1 = np.ascontiguousarray(np.concatenate(
        [as1.reshape(-1), ad1.reshape(-1)]).reshape(1, -1).astype(np.float32))
    ins1 = []
    for c in range(NC):
        xTp = np.zeros((P, NPAD), dtype=ml_dtypes.bfloat16)
        xTp[:, :NPC] = x[c * NPC:(c + 1) * NPC, :].T.astype(ml_dtypes.bfloat16)
        ins1.append({n1['xT']: xTp, n1['W1']: W1, n1['att1']: att1})
    r1 = bass_utils.run_bass_kernel_spmd(nc1, ins1, core_ids=list(range(NC)),
                                         trace=trace)
    times['dense1'] = r1.exec_time_ns
    t1_shards = [r1.results[c][n1['T1']] for c in range(NC)]
    T1full = np.ascontiguousarray(
        np.concatenate([t[:NPC] for t in t1_shards]))
    ad1_shards = []
    for c in range(NC):
        a = np.zeros((NPAD, 8), np.float32)
        a[:NPC] = np.ascontiguousarray(
            t1_shards[c][:NPC, 272:288]).view(np.float32)
        ad1_shards.append(a)

    nc2, n2 = build_edge(1, plan, dims)
    att2 = np.ascontiguousarray(np.concatenate(
        [as2.reshape(-1), ad2.reshape(-1)]).reshape(1, -1).astype(np.float32))
    ins2 = [{n2['G']: T1full, n2['AD']: ad1_shards[c], n2['W2']: W2,
             n2['att2']: att2,
             n2['bias']: np.ascontiguousarray(b1.reshape(1, -1)),
             n2['gidx']: streams[c]['gidx'],
             n2['dstloc']: streams[c]['dstloc'],
             n2['dstlocT']: streams[c]['dstlocT']} for c in range(NC)]
    r2 = bass_utils.run_bass_kernel_spmd(nc2, ins2, core_ids=list(range(NC)),
                                         trace=trace)
    times['edge1'] = r2.exec_time_ns
    t2_shards = [r2.results[c][n2['T2']] for c in range(NC)]
    T2full = np.ascontiguousarray(
        np.concatenate([t[:NPC] for t in t2_shards]))
    ad2_shards = []
    for c in range(NC):
        a = np.zeros((NPAD, 8), np.float32)
        a[:NPC] = t2_shards[c][:NPC, 136:144].astype(np.float32)
        ad2_shards.append(a)

    nc3, n3 = build_edge(2, plan, dims)
    ins3 = [{n3['G']: T2full, n3['AD']: ad2_shards[c],
             n3['bias']: np.ascontiguousarray(b2.reshape(1, -1)),
             n3['gidx']: streams[c]['gidx'],
             n3['dstloc']: streams[c]['dstloc'],
             n3['dstlocT']: streams[c]['dstlocT']} for c in range(NC)]
    r3 = bass_utils.run_bass_kernel_spmd(nc3, ins3, core_ids=list(range(NC)),
                                         trace=trace)
    times['edge2'] = r3.exec_time_ns
    out = np.concatenate([r3.results[c][n3['OUT']][:NPC] for c in range(NC)])
    return np.ascontiguousarray(out.astype(np.float32)), times


def kernel(**inputs):
    out, _ = _run_pipeline(inputs, Dims(), trace=False)
    return out


# revision 22
# speedup vs baseline: 1.2145x; 1.0023x over previous
"""Self-contained Trainium2 Bass kernel for the 2-layer GAT
(nn_GAT_18915035971953): 100000 nodes, 1.6M edges, 8 NeuronCores.

Strategy: edges sorted by destination and dst-sharded across 8 cores
(12500 dst nodes each). Per 128-dst window, per-edge source rows are
fetched with dma_gather; gathers are batched over GSW=4 windows per
src-bucket. Per-edge a_d is produced without per-slot transposes: a
1-row matmul broadcasts the transposed dst-slot stream across
partitions, one is_equal builds the transposed one-hot (fp8), and
8-wide matmuls select a_d per edge. Edge softmax weights are computed
on-chip; the segment sum is a one-hot matmul into PSUM. Layer-1 node
tables pack h1 + a_s + a_d in one row (gathered at 576B); layer 2
gathers precomputed h2 = relu(out1+b1) @ W2 rows with a_s2/a_d2
packed. Host extracts per-dst a_d tables from the row pads between
launches (index ops only). Windows are software-pipelined (build
stage w+2 while aggregating w), outputs are written once per
supergroup. Three SPMD launches: dense tables -> edge1 -> edge2."""
import sys
from dataclasses import dataclass
import numpy as np
import ml_dtypes

if "/opt/trn_rl_repo" not in sys.path:
    sys.path.insert(0, "/opt/trn_rl_repo")

import concourse.bacc as bacc
import concourse.mybir as mybir
import concourse.tile as tile
from concourse.masks import make_identity
from concourse import bass_utils

# ---------------- constants ----------------

P = 128
H = 8
NQ = 4            # SWDGE queues
GSW = 3           # windows per gather supergroup
MAXNIDX = 1024    # max idxs per dma_gather call (HW limit)
F32 = mybir.dt.float32
BF16 = mybir.dt.bfloat16
FP8 = mybir.dt.float8e4
I16 = mybir.dt.int16
AF = mybir.ActivationFunctionType
ALU = mybir.AluOpType
AX = mybir.AxisListType


@dataclass
class Dims:
    N: int = 100000
    NCORES: int = 8
    NBUCK: int = 4           # src buckets (int16 gather indices < 32768)

    @property
    def NPC(self):
        return self.N // self.NCORES

    @property
    def BUCK(self):
        return self.N // self.NBUCK

    @property
    def NWIN(self):
        return (self.NPC + P - 1) // P

    @property
    def NPAD(self):
        return self.NWIN * P


# ---------------- host-side index preprocessing ----------------


def _wrap16(idx):
    n = idx.shape[0]
    assert n % 16 == 0
    w = idx.reshape(n // 16, 16).T.astype(np.int16)
    return np.tile(w, (8, 1))


def build_plans(edge_index, dims: Dims):
    """Slot layout: for each supergroup sw (GSW windows), for each src
    bucket b, the (w, b) segments of sw's windows are concatenated (each
    padded to a 128 multiple) and fetched with ONE dma_gather call
    (split at MAXNIDX).

    Returns (plan, streams).
    plan: dict(supers=[{S, slot0, calls=[(b, n, gcol0, scol)],
                        windows=[w...]}],
               windows=[{w, segs=[(scol, ns)]}], slots, gcols)
      scol is slot offset LOCAL to the supergroup.
    streams (per core): gidx int16 [128, gcols], dstloc bf16 [128, slots],
      dstlocT bf16 [1, slots*128] (edge-order dst-slot values).
    Pad slots: gather row 0 of the bucket (finite data), dstloc = -1
    (one-hot row all-zero -> edge excluded from aggregation)."""
    N, NC, NB, BUCK = dims.N, dims.NCORES, dims.NBUCK, dims.BUCK
    NPC, NWIN = dims.NPC, dims.NWIN
    src = np.asarray(edge_index[0], np.int64)
    dst = np.asarray(edge_index[1], np.int64)
    order = np.argsort(dst, kind="stable")
    s_src, s_dst = src[order], dst[order]
    counts = np.bincount(s_dst, minlength=N)
    node_start = np.concatenate([[0], np.cumsum(counts)])

    seg = [[[None] * NB for _ in range(NWIN)] for _ in range(NC)]
    for c in range(NC):
        d0 = c * NPC
        for w in range(NWIN):
            lo = node_start[d0 + w * P]
            hi = node_start[min(d0 + (w + 1) * P, d0 + NPC)]
            esrc = s_src[lo:hi]
            edst = s_dst[lo:hi]
            for b in range(NB):
                m = (esrc // BUCK) == b
                seg[c][w][b] = (esrc[m] - b * BUCK, edst[m] - (d0 + w * P))

    nn = np.zeros((NWIN, NB), np.int64)
    for w in range(NWIN):
        for b in range(NB):
            kmax = max(seg[c][w][b][0].shape[0] for c in range(NC))
            nn[w, b] = ((kmax + P - 1) // P) * P
    for w in range(NWIN):
        if nn[w].sum() == 0:
            nn[w, 0] = P

    supers = []
    windows = [dict(w=w, segs=[]) for w in range(NWIN)]
    core_g = [[] for _ in range(NC)]
    core_dl = [[] for _ in range(NC)]
    core_dlT = [[] for _ in range(NC)]
    gcol0 = 0
    slot0 = 0
    for sw0 in range(0, NWIN, GSW):
        ws = list(range(sw0, min(sw0 + GSW, NWIN)))
        calls = []
        scol = 0
        for b in range(NB):
            nb_tot = int(sum(nn[w, b] for w in ws))
            if nb_tot == 0:
                continue
            for w in ws:
                k = int(nn[w, b])
                if k == 0:
                    continue
                windows[w]['segs'].append((scol, k // P))
                for c in range(NC):
                    es, ed = seg[c][w][b]
                    kk = es.shape[0]
                    gi = np.concatenate([es, np.zeros(k - kk, np.int64)])
                    dl = np.concatenate([ed, np.full(k - kk, -1, np.int64)])
                    core_g[c].append(_wrap16(gi))
                    core_dl[c].append(dl.reshape(k // P, P).T)
                    core_dlT[c].append(dl)
                scol += k // P
            off = 0
            while off < nb_tot:
                take = min(MAXNIDX, nb_tot - off)
                calls.append((b, take, gcol0 + off // 16,
                              (scol - nb_tot // P) + off // P))
                off += take
            gcol0 += nb_tot // 16
        supers.append(dict(S=scol, slot0=slot0, calls=calls, windows=ws))
        slot0 += scol

    plan = dict(supers=supers, windows=windows, slots=slot0, gcols=gcol0)
    streams = []
    for c in range(NC):
        streams.append(dict(
            gidx=np.ascontiguousarray(np.concatenate(core_g[c], axis=1)),
            dstloc=np.ascontiguousarray(
                np.concatenate(core_dl[c], axis=1).astype(ml_dtypes.bfloat16)),
            dstlocT=np.ascontiguousarray(
                np.concatenate(core_dlT[c]).reshape(1, -1)
                .astype(ml_dtypes.bfloat16)),
        ))
    return plan, streams


# ---------------- kernel builders ----------------


def build_dense1(dims: Dims):
    """h1 = x @ W1 (bf16 matmul) + attention folds. Output rows
    T1 [NPAD, 384] bf16 (768B rows): cols 0:256 h1 feats, 256:272 a_s
    (bitcast f32), 272:288 a_d (bitcast f32).
    Host extracts the per-dst a_d table. xT input is host-padded to
    NPAD columns (zeros past NPC)."""
    NPAD = dims.NPAD
    NWIN = dims.NWIN
    nc = bacc.Bacc(None, target_bir_lowering=False, num_swdge_queues=NQ)
    with tile.TileContext(nc) as tc:
        with tc.tile_pool(name="dram", bufs=1, space="DRAM") as dram:
            xT = dram.tile([P, NPAD], BF16, kind="ExternalInput")
            W1 = dram.tile([P, 256], F32, kind="ExternalInput")
            att1 = dram.tile([1, 512], F32, kind="ExternalInput")
            T1 = dram.tile([NPAD, 384], BF16, kind="ExternalOutput")
            names = dict(xT=xT.name, W1=W1.name, att1=att1.name, T1=T1.name)
            with tc.tile_pool(name="cst", bufs=1) as cst, \
                 tc.tile_pool(name="wk", bufs=3) as wk, \
                 tc.tile_pool(name="ps", bufs=4, space="PSUM") as ps:
                xTs = cst.tile([P, NPAD], BF16)
                nc.sync.dma_start(xTs[:], xT[:])
                W1s = cst.tile([P, 256], F32)
                nc.sync.dma_start(W1s[:], W1[:])
                att_s = cst.tile([1, 512], F32)
                nc.sync.dma_start(att_s[:], att1[:])
                attb = cst.tile([P, 512], F32)
                nc.gpsimd.partition_broadcast(attb[:, 0:256], att_s[0:1, 0:256])
                nc.gpsimd.partition_broadcast(attb[:, 256:512], att_s[0:1, 256:512])
                tmp = cst.tile([P, 512], F32)
                nc.vector.tensor_tensor(out=tmp[:, 0:256], in0=W1s[:],
                                        in1=attb[:, 0:256], op=ALU.mult)
                nc.vector.tensor_tensor(out=tmp[:, 256:512], in0=W1s[:],
                                        in1=attb[:, 256:512], op=ALU.mult)
                tv = tmp[:].rearrange("p (v h f) -> p v h f", v=2, h=H)
                folds = cst.tile([P, 16], F32)
                nc.vector.tensor_reduce(out=folds[:, 0:8], in_=tv[:, 0],
                                        axis=AX.X, op=ALU.add)
                nc.vector.tensor_reduce(out=folds[:, 8:16], in_=tv[:, 1],
                                        axis=AX.X, op=ALU.add)
                rhs = cst.tile([P, 272], BF16)
                nc.vector.tensor_copy(rhs[:, 0:256], W1s[:])
                nc.vector.tensor_copy(rhs[:, 256:272], folds[:])
                grp = 0
                while grp * GSW < NWIN:
                    ws = list(range(grp * GSW, min((grp + 1) * GSW, NWIN)))
                    nw = len(ws)
                    t14 = wk.tile([P, nw, 288], BF16, tag="t14")
                    for j, w in enumerate(ws):
                        po = ps.tile([P, 272], F32, tag="po")
                        nc.tensor.matmul(out=po[:],
                                         lhsT=xTs[:, w * P:(w + 1) * P],
                                         rhs=rhs[:], start=True, stop=True)
                        nc.scalar.copy(t14[:, j, 0:256], po[:, 0:256])
                        nc.vector.tensor_copy(
                            t14[:, j, 256:288].bitcast(F32)[:, 0:16],
                            po[:, 256:272])
                    nc.sync.dma_start(
                        T1[ws[0] * P:(ws[-1] + 1) * P, 0:288]
                        .rearrange("(w p) c -> p w c", p=P), t14[:])
                    grp += 1
    nc.compile()
    return nc, names


def build_edge(layer, plan, dims: Dims):
    """Edge phase for layer 1 or 2.

    layer 1: gathers T1 rows (576B: h1 256 bf16 + a_s/a_d bitcast f32),
      aggregates per-head h1*alpha (264-col one-hot matmul), outputs
      T2 [NPAD, 256] bf16 rows: 0:128 h2 = relu(out1+b1)@W2 per head,
      128:136 a_s2 bf16, 136:144 a_d2 bf16.
    layer 2: gathers T2 rows (512B), aggregates per-head h2*alpha
      (136-col matmul), outputs OUT [NPAD, 16] f32.

    Per-edge a_d: 1-row matmul broadcasts the transposed dst-slot
    stream; is_equal vs the partition index builds the transposed
    one-hot (fp8); 8-wide matmuls with the fp8 a_d table select a_d."""
    N, BUCK, NWIN, NPAD = dims.N, dims.BUCK, dims.NWIN, dims.NPAD
    supers, windows = plan['supers'], plan['windows']
    gcols, slots = plan['gcols'], plan['slots']
    GELEM = 384 if layer == 1 else 256   # gathered elements per row
    GSTEP = 384 if layer == 1 else 256   # table row stride (elements)
    FW = 256 if layer == 1 else 128      # feature width
    AGG = FW + 8
    OW = 144 if layer == 1 else 16       # output row width
    nc = bacc.Bacc(None, target_bir_lowering=False, num_swdge_queues=NQ)
    qctr = [0]

    def nextq():
        q = qctr[0] % NQ
        qctr[0] += 1
        return q

    with tile.TileContext(nc) as tc:
        with tc.tile_pool(name="dram", bufs=1, space="DRAM") as dram:
            names = {}
            Gt = dram.tile([N, GSTEP], BF16, kind="ExternalInput")
            ADt = dram.tile([NPAD, 8], F32, kind="ExternalInput")
            nb = 32 if layer == 1 else 16
            bias = dram.tile([1, nb], F32, kind="ExternalInput")
            gidx = dram.tile([P, gcols], I16, kind="ExternalInput")
            dstloc = dram.tile([P, slots], BF16, kind="ExternalInput")
            dstlocT = dram.tile([1, slots * P], BF16, kind="ExternalInput")
            names.update(G=Gt.name, AD=ADt.name, bias=bias.name,
                         gidx=gidx.name, dstloc=dstloc.name,
                         dstlocT=dstlocT.name)
            if layer == 1:
                W2 = dram.tile([32, 128], F32, kind="ExternalInput")
                att2 = dram.tile([1, 256], F32, kind="ExternalInput")
                T2o = dram.tile([NPAD, 256], BF16, kind="ExternalOutput")
                names.update(W2=W2.name, att2=att2.name, T2=T2o.name)
            else:
                OUT = dram.tile([NPAD, 16], F32, kind="ExternalOutput")
                names.update(OUT=OUT.name)

            with tc.tile_pool(name="cst", bufs=1) as cst, \
                 tc.tile_pool(name="gp", bufs=2) as gp, \
                 tc.tile_pool(name="gi", bufs=2) as gip, \
                 tc.tile_pool(name="gd", bufs=1) as gdp, \
                 tc.tile_pool(name="wk1", bufs=8) as wk1, \
                 tc.tile_pool(name="wk2", bufs=4) as wk2, \
                 tc.tile_pool(name="wk3", bufs=3) as wk3, \
                 tc.tile_pool(name="wo", bufs=2) as wo, \
                 tc.tile_pool(name="psa", bufs=2, space="PSUM") as psa, \
                 tc.tile_pool(name="psb", bufs=2, space="PSUM") as psb, \
                 tc.tile_pool(name="pso", bufs=1, space="PSUM") as pso, \
                 tc.tile_pool(name="psd", bufs=2, space="PSUM") as psd:
                dstloc_s = cst.tile([P, slots], BF16)
                nc.sync.dma_start(dstloc_s[:], dstloc[:])
                ad_s = cst.tile([P, NWIN, 8], F32)
                nc.sync.dma_start(
                    ad_s[:], ADt[:].rearrange("(w p) c -> p w c", p=P))
                ad8 = cst.tile([P, NWIN, 8], FP8)
                nc.vector.tensor_copy(ad8[:], ad_s[:])
                iota_i = cst.tile([P, P], mybir.dt.int32)
                nc.gpsimd.iota(iota_i[:], pattern=[[1, P]], base=0,
                               channel_multiplier=0)
                iota_bf = cst.tile([P, P], BF16)
                nc.vector.tensor_copy(iota_bf[:], iota_i[:])
                iota_pi = cst.tile([P, 1], mybir.dt.int32)
                nc.gpsimd.iota(iota_pi[:], pattern=[[0, 1]], base=0,
                               channel_multiplier=1)
                iota_pb = cst.tile([P, 1], BF16)
                nc.vector.tensor_copy(iota_pb[:], iota_pi[:])
                ones1 = cst.tile([1, P], BF16)
                nc.vector.memset(ones1[:], 1.0)
                bias_s = cst.tile([1, nb], F32)
                nc.sync.dma_start(bias_s[:], bias[:])
                bias_b = cst.tile([P, nb], F32)
                nc.gpsimd.partition_broadcast(bias_b[:], bias_s[0:1, :])
                if layer == 1:
                    W2s = cst.tile([32, 128], F32)
                    nc.sync.dma_start(W2s[:], W2[:])
                    att2_s = cst.tile([1, 256], F32)
                    nc.sync.dma_start(att2_s[:], att2[:])
                    att2b = cst.tile([32, 256], F32)
                    nc.gpsimd.partition_broadcast(att2b[:, 0:128],
                                                  att2_s[0:1, 0:128])
                    nc.gpsimd.partition_broadcast(att2b[:, 128:256],
                                                  att2_s[0:1, 128:256])
                    tmp2 = cst.tile([32, 256], F32)
                    nc.vector.tensor_tensor(out=tmp2[:, 0:128], in0=W2s[:],
                                            in1=att2b[:, 0:128], op=ALU.mult)
                    nc.vector.tensor_tensor(out=tmp2[:, 128:256], in0=W2s[:],
                                            in1=att2b[:, 128:256], op=ALU.mult)
                    t2v = tmp2[:].rearrange("p (v h f) -> p v h f", v=2, h=H)
                    W2cat = cst.tile([32, 144], F32)
                    nc.vector.tensor_copy(W2cat[:, 0:128], W2s[:])
                    nc.vector.tensor_reduce(out=W2cat[:, 128:136],
                                            in_=t2v[:, 0], axis=AX.X,
                                            op=ALU.add)
                    nc.vector.tensor_reduce(out=W2cat[:, 136:144],
                                            in_=t2v[:, 1], axis=AX.X,
                                            op=ALU.add)
                    ident = cst.tile([P, P], F32)
                    make_identity(nc, ident[:])

                for sup in supers:
                    S = sup['S']
                    ncols = sum(n for (_, n, _, _) in sup['calls']) // 16
                    gc_base = sup['calls'][0][2]
                    gidx_t = gip.tile([P, ncols], I16, tag="gi")
                    nc.sync.dma_start(gidx_t[:],
                                      gidx[:, gc_base:gc_base + ncols])
                    dlT_t = gdp.tile([1, S * P], BF16, tag="dlt")
                    nc.sync.dma_start(
                        dlT_t[:],
                        dstlocT[0:1, sup['slot0'] * P:(sup['slot0'] + S) * P])
                    g_t = gp.tile([P, S, GELEM], BF16, tag="g")
                    for (b, n, gc0, scol) in sup['calls']:
                        nc.gpsimd.dma_gather(
                            g_t[:, scol:scol + n // P, :],
                            Gt[b * BUCK:(b + 1) * BUCK, 0:GELEM],
                            gidx_t[:, gc0 - gc_base:gc0 - gc_base + n // 16],
                            n, n, GELEM, elem_step=GSTEP, queue_num=nextq())
                    ws = sup['windows']
                    nw = len(ws)
                    out_t = wo.tile([P, nw, OW], BF16 if layer == 1 else F32,
                                    tag="out")
                    built = {}

                    def loop1(w):
                        segs = windows[w]['segs']
                        tiles = []
                        for (scol, ns) in segs:
                            pts = wk2.tile([P, ns * P], FP8, tag="pts")
                            for c0 in range(0, ns, 4):
                                cn = min(4, ns - c0)
                                pb = psb.tile([P, cn * P], F32, tag="pb")
                                nc.tensor.matmul(
                                    out=pb[:], lhsT=ones1[:],
                                    rhs=dlT_t[0:1, (scol + c0) * P:
                                              (scol + c0 + cn) * P],
                                    start=True, stop=True)
                                nc.vector.tensor_tensor(
                                    out=pts[:, c0 * P:(c0 + cn) * P],
                                    in0=pb[:],
                                    in1=iota_pb[:].to_broadcast([P, cn * P]),
                                    op=ALU.is_equal)
                            adE = psd.tile([P, ns * 8], F32, tag="adE")
                            for k in range(ns):
                                nc.tensor.matmul(
                                    out=adE[:, k * 8:(k + 1) * 8],
                                    lhsT=pts[:, k * P:(k + 1) * P],
                                    rhs=ad8[:, w, :], start=True, stop=True)
                            p_t = wk1.tile([P, ns, P], BF16, tag="pt")
                            dl_b = dstloc_s[:, sup['slot0'] + scol:
                                            sup['slot0'] + scol + ns] \
                                .unsqueeze(2).to_broadcast([P, ns, P])
                            io_b = iota_bf[:].unsqueeze(1) \
                                .to_broadcast([P, ns, P])
                            nc.vector.tensor_tensor(out=p_t[:], in0=dl_b,
                                                    in1=io_b, op=ALU.is_equal)
                            if layer == 1:
                                a_s_ap = g_t[:] \
                                    .rearrange("p s e -> p (s e)") \
                                    .bitcast(F32) \
                                    .rearrange("p (s e) -> p s e", e=192) \
                                    [:, scol:scol + ns, 128:136]
                            else:
                                a_s_ap = g_t[:, scol:scol + ns, 128:136]
                            et = wk2.tile([P, ns, 8], F32, tag="et")
                            nc.vector.tensor_tensor(
                                out=et[:], in0=a_s_ap,
                                in1=adE[:].rearrange("p (s e) -> p s e", e=8),
                                op=ALU.add)
                            nc.vector.scalar_tensor_tensor(
                                out=et[:], in0=et[:], scalar=0.2, in1=et[:],
                                op0=ALU.mult, op1=ALU.max)
                            rhs_t = wk1.tile([P, ns, AGG], BF16, tag="rhs")
                            nc.scalar.activation(rhs_t[:, :, FW:FW + 8],
                                                 et[:], AF.Exp)
                            wexp_b = rhs_t[:, :, FW:FW + 8].unsqueeze(3) \
                                .to_broadcast([P, ns, 8, FW // 8])
                            g_v = g_t[:, scol:scol + ns, 0:FW] \
                                .rearrange("p s (h f) -> p s h f", h=H)
                            nc.vector.tensor_tensor(
                                out=rhs_t[:, :, 0:FW]
                                .rearrange("p s (h f) -> p s h f", h=H),
                                in0=g_v, in1=wexp_b, op=ALU.mult)
                            tiles.append((p_t, rhs_t, ns))
                        built[w] = tiles

                    def loop2(w, wi):
                        tiles = built.pop(w)
                        nslot = sum(ns for (_, _, ns) in tiles)
                        agg = psa.tile([P, AGG], F32, tag="agg")
                        sdone = 0
                        for (p_t, rhs_t, ns) in tiles:
                            for k in range(ns):
                                nc.tensor.matmul(
                                    out=agg[:], lhsT=p_t[:, k, :],
                                    rhs=rhs_t[:, k, :],
                                    start=(sdone + k == 0),
                                    stop=(sdone + k == nslot - 1))
                            sdone += ns
                        zr = wk3.tile([P, 8], F32, tag="zr")
                        nc.vector.tensor_scalar_add(zr[:], agg[:, FW:FW + 8],
                                                    1e-16)
                        nc.vector.reciprocal(zr[:], zr[:])
                        nc.vector.tensor_scalar_mul(zr[:], zr[:], 1.0 / H)
                        zrb = zr[:].unsqueeze(2).to_broadcast([P, H, FW // 8])
                        hn = wk3.tile([P, FW], F32, tag="hn")
                        nc.vector.tensor_tensor(
                            out=hn[:].rearrange("p (h f) -> p h f", h=H),
                            in0=agg[:, 0:FW].rearrange("p (h f) -> p h f", h=H),
                            in1=zrb, op=ALU.mult)
                        if layer == 1:
                            o1 = wk3.tile([P, 32], F32, tag="o1")
                            nc.vector.tensor_reduce(
                                out=o1[:],
                                in_=hn[:].rearrange("p (h f) -> p f h", h=H),
                                axis=AX.X, op=ALU.add)
                            nc.vector.tensor_tensor(out=o1[:], in0=o1[:],
                                                    in1=bias_b[:, 0:32],
                                                    op=ALU.add)
                            nc.vector.tensor_scalar_max(o1[:], o1[:], 0.0)
                            hT = pso.tile([32, P], F32, tag="hT")
                            nc.tensor.transpose(hT[:], o1[:], ident[:])
                            hTs = wk3.tile([32, P], F32, tag="hTs")
                            nc.vector.tensor_copy(hTs[:], hT[:])
                            h2a = pso.tile([P, 144], F32, tag="h2a")
                            nc.tensor.matmul(out=h2a[:], lhsT=hTs[:],
                                             rhs=W2cat[:], start=True,
                                             stop=True)
                            nc.scalar.copy(out_t[:, wi, :], h2a[:])
                        else:
                            nc.vector.tensor_reduce(
                                out=out_t[:, wi, :],
                                in_=hn[:].rearrange("p (h f) -> p f h", h=H),
                                axis=AX.X, op=ALU.add)
                            nc.vector.tensor_tensor(out=out_t[:, wi, :],
                                                    in0=out_t[:, wi, :],
                                                    in1=bias_b[:, 0:16],
                                                    op=ALU.add)

                    prev = []
                    for w in ws:
                        loop1(w)
                        prev.append(w)
                        if len(prev) > 1:
                            loop2(prev[0], ws.index(prev[0]))
                            prev.pop(0)
                    for w in prev:
                        loop2(w, ws.index(w))
                    dst_ap = (T2o if layer == 1 else OUT)
                    nc.sync.dma_start(
                        dst_ap[ws[0] * P:(ws[-1] + 1) * P, 0:OW]
                        .rearrange("(w p) c -> p w c", p=P), out_t[:])
    nc.compile()
    return nc, names


# ---------------- driver ----------------


def _run_pipeline(inputs, dims, trace=False):
    x = np.asarray(inputs['x'], np.float32)
    ei = np.asarray(inputs['edge_index'])
    W1 = np.ascontiguousarray(np.asarray(inputs['W1'], np.float32))
    as1 = np.asarray(inputs['att_src1'], np.float32)
    ad1 = np.asarray(inputs['att_dst1'], np.float32)
    b1 = np.asarray(inputs['b1'], np.float32)
    W2 = np.ascontiguousarray(np.asarray(inputs['W2'], np.float32))
    as2 = np.asarray(inputs['att_src2'], np.float32)
    ad2 = np.asarray(inputs['att_dst2'], np.float32)
    b2 = np.asarray(inputs['b2'], np.float32)
    NC, NPC, NPAD = dims.NCORES, dims.NPC, dims.NPAD

    plan, streams = build_plans(ei, dims)
    times = {}

    nc1, n1 = build_dense1(dims)
    att1 = np.ascontiguousarray(np.concatenate(
        [as1.reshape(-1), ad1.reshape(-1)]).reshape(1, -1).astype(np.float32))
    ins1 = []
    for c in range(NC):
        xTp = np.zeros((P, NPAD), dtype=ml_dtypes.bfloat16)
        xTp[:, :NPC] = x[c * NPC:(c + 1) * NPC, :].T.astype(ml_dtypes.bfloat16)
        ins1.append({n1['xT']: xTp, n1['W1']: W1, n1['att1']: att1})
    r1 = bass_utils.run_bass_kernel_spmd(nc1, ins1, core_ids=list(range(NC)),
                                         trace=trace)
    times['dense1'] = r1.exec_time_ns
    t1_shards = [r1.results[c][n1['T1']] for c in range(NC)]
    T1full = np.ascontiguousarray(
        np.concatenate([t[:NPC] for t in t1_shards]))
    ad1_shards = []
    for c in range(NC):
        a = np.zeros((NPAD, 8), np.float32)
        a[:NPC] = np.ascontiguousarray(
            t1_shards[c][:NPC, 272:288]).view(np.float32)
        ad1_shards.append(a)

    nc2, n2 = build_edge(1, plan, dims)
    att2 = np.ascontiguousarray(np.concatenate(
        [as2.reshape(-1), ad2.reshape(-1)]).reshape(1, -1).astype(np.float32))
    ins2 = [{n2['G']: T1full, n2['AD']: ad1_shards[c], n2['W2']: W2,
             n2['att2']: att2,
             n2['bias']: np.ascontiguousarray(b1.reshape(1, -1)),
             n2['gidx']: streams[c]['gidx'],
             n2['dstloc']: streams[c]['dstloc'],
             n2['dstlocT']: streams[c]['dstlocT']} for c in range(NC)]
    r2 = bass_utils.run_bass_kernel_spmd(nc2, ins2, core_ids=list(range(NC)),
                                         trace=trace)
    times['edge1'] = r2.exec_time_ns
    t2_shards = [r2.results[c][n2['T2']] for c in range(NC)]
    T2full = np.ascontiguousarray(
        np.concatenate([t[:NPC] for t in t2_shards]))
    ad2_shards = []
    for c in range(NC):
        a = np.zeros((NPAD, 8), np.float32)
        a[:NPC] = t2_shards[c][:NPC, 136:144].astype(np.float32)
        ad2_shards.append(a)

    nc3, n3 = build_edge(2, plan, dims)
    ins3 = [{n3['G']: T2full, n3['AD']: ad2_shards[c],
             n3['bias']: np.ascontiguousarray(b2.reshape(1, -1)),
             n3['gidx']: streams[c]['gidx'],
             n3['dstloc']: streams[c]['dstloc'],
             n3['dstlocT']: streams[c]['dstlocT']} for c in range(NC)]
    r3 = bass_utils.run_bass_kernel_spmd(nc3, ins3, core_ids=list(range(NC)),
                                         trace=trace)
    times['edge2'] = r3.exec_time_ns
    out = np.concatenate([r3.results[c][n3['OUT']][:NPC] for c in range(NC)])
    return np.ascontiguousarray(out.astype(np.float32)), times


def kernel(**inputs):
    out, _ = _run_pipeline(inputs, Dims(), trace=False)
    return out


# revision 23
# speedup vs baseline: 1.2310x; 1.0136x over previous
"""Self-contained Trainium2 Bass kernel for the 2-layer GAT
(nn_GAT_18915035971953): 100000 nodes, 1.6M edges, 8 NeuronCores.

Strategy: edges sorted by destination and dst-sharded across 8 cores
(12500 dst nodes each). Per 128-dst window, per-edge source rows are
fetched with dma_gather; gathers are batched over GSW=4 windows per
src-bucket. Per-edge a_d is produced without per-slot transposes: a
1-row matmul broadcasts the transposed dst-slot stream across
partitions, one is_equal builds the transposed one-hot (fp8), and
8-wide matmuls select a_d per edge. Edge softmax weights are computed
on-chip; the segment sum is a one-hot matmul into PSUM. Layer-1 node
tables pack h1 + a_s + a_d in one row (gathered at 576B); layer 2
gathers precomputed h2 = relu(out1+b1) @ W2 rows with a_s2/a_d2
packed. Host extracts per-dst a_d tables from the row pads between
launches (index ops only). Windows are software-pipelined (build
stage w+2 while aggregating w), outputs are written once per
supergroup. Three SPMD launches: dense tables -> edge1 -> edge2."""
import sys
from dataclasses import dataclass
import numpy as np
import ml_dtypes

if "/opt/trn_rl_repo" not in sys.path:
    sys.path.insert(0, "/opt/trn_rl_repo")

import concourse.bacc as bacc
import concourse.mybir as mybir
import concourse.tile as tile
from concourse.masks import make_identity
from concourse import bass_utils

# ---------------- constants ----------------

P = 128
H = 8
NQ = 4            # SWDGE queues
GSW = 3           # windows per gather supergroup
MAXNIDX = 1024    # max idxs per dma_gather call (HW limit)
F32 = mybir.dt.float32
BF16 = mybir.dt.bfloat16
FP8 = mybir.dt.float8e4
I16 = mybir.dt.int16
AF = mybir.ActivationFunctionType
ALU = mybir.AluOpType
AX = mybir.AxisListType


@dataclass
class Dims:
    N: int = 100000
    NCORES: int = 8
    NBUCK: int = 4           # src buckets (int16 gather indices < 32768)

    @property
    def NPC(self):
        return self.N // self.NCORES

    @property
    def BUCK(self):
        return self.N // self.NBUCK

    @property
    def NWIN(self):
        return (self.NPC + P - 1) // P

    @property
    def NPAD(self):
        return self.NWIN * P


# ---------------- host-side index preprocessing ----------------


def _wrap16(idx):
    n = idx.shape[0]
    assert n % 16 == 0
    w = idx.reshape(n // 16, 16).T.astype(np.int16)
    return np.tile(w, (8, 1))


def build_plans(edge_index, dims: Dims):
    """Slot layout: for each supergroup sw (GSW windows), for each src
    bucket b, the (w, b) segments of sw's windows are concatenated (each
    padded to a 128 multiple) and fetched with ONE dma_gather call
    (split at MAXNIDX).

    Returns (plan, streams).
    plan: dict(supers=[{S, slot0, calls=[(b, n, gcol0, scol)],
                        windows=[w...]}],
               windows=[{w, segs=[(scol, ns)]}], slots, gcols)
      scol is slot offset LOCAL to the supergroup.
    streams (per core): gidx int16 [128, gcols], dstloc bf16 [128, slots],
      dstlocT bf16 [1, slots*128] (edge-order dst-slot values).
    Pad slots: gather row 0 of the bucket (finite data), dstloc = -1
    (one-hot row all-zero -> edge excluded from aggregation)."""
    N, NC, NB, BUCK = dims.N, dims.NCORES, dims.NBUCK, dims.BUCK
    NPC, NWIN = dims.NPC, dims.NWIN
    src = np.asarray(edge_index[0], np.int64)
    dst = np.asarray(edge_index[1], np.int64)
    order = np.argsort(dst, kind="stable")
    s_src, s_dst = src[order], dst[order]
    counts = np.bincount(s_dst, minlength=N)
    node_start = np.concatenate([[0], np.cumsum(counts)])

    seg = [[[None] * NB for _ in range(NWIN)] for _ in range(NC)]
    for c in range(NC):
        d0 = c * NPC
        for w in range(NWIN):
            lo = node_start[d0 + w * P]
            hi = node_start[min(d0 + (w + 1) * P, d0 + NPC)]
            esrc = s_src[lo:hi]
            edst = s_dst[lo:hi]
            for b in range(NB):
                m = (esrc // BUCK) == b
                seg[c][w][b] = (esrc[m] - b * BUCK, edst[m] - (d0 + w * P))

    nn = np.zeros((NWIN, NB), np.int64)
    for w in range(NWIN):
        for b in range(NB):
            kmax = max(seg[c][w][b][0].shape[0] for c in range(NC))
            nn[w, b] = ((kmax + P - 1) // P) * P
    for w in range(NWIN):
        if nn[w].sum() == 0:
            nn[w, 0] = P

    supers = []
    windows = [dict(w=w, segs=[]) for w in range(NWIN)]
    core_g = [[] for _ in range(NC)]
    core_dl = [[] for _ in range(NC)]
    core_dlT = [[] for _ in range(NC)]
    gcol0 = 0
    slot0 = 0
    for sw0 in range(0, NWIN, GSW):
        ws = list(range(sw0, min(sw0 + GSW, NWIN)))
        calls = []
        scol = 0
        for b in range(NB):
            nb_tot = int(sum(nn[w, b] for w in ws))
            if nb_tot == 0:
                continue
            for w in ws:
                k = int(nn[w, b])
                if k == 0:
                    continue
                windows[w]['segs'].append((scol, k // P))
                for c in range(NC):
                    es, ed = seg[c][w][b]
                    kk = es.shape[0]
                    gi = np.concatenate([es, np.zeros(k - kk, np.int64)])
                    dl = np.concatenate([ed, np.full(k - kk, -1, np.int64)])
                    core_g[c].append(_wrap16(gi))
                    core_dl[c].append(dl.reshape(k // P, P).T)
                    core_dlT[c].append(dl)
                scol += k // P
            off = 0
            while off < nb_tot:
                take = min(MAXNIDX, nb_tot - off)
                calls.append((b, take, gcol0 + off // 16,
                              (scol - nb_tot // P) + off // P))
                off += take
            gcol0 += nb_tot // 16
        supers.append(dict(S=scol, slot0=slot0, calls=calls, windows=ws))
        slot0 += scol

    plan = dict(supers=supers, windows=windows, slots=slot0, gcols=gcol0)
    streams = []
    for c in range(NC):
        streams.append(dict(
            gidx=np.ascontiguousarray(np.concatenate(core_g[c], axis=1)),
            dstloc=np.ascontiguousarray(
                np.concatenate(core_dl[c], axis=1).astype(ml_dtypes.bfloat16)),
            dstlocT=np.ascontiguousarray(
                np.concatenate(core_dlT[c]).reshape(1, -1)
                .astype(ml_dtypes.bfloat16)),
        ))
    return plan, streams


# ---------------- kernel builders ----------------


def build_dense1(dims: Dims):
    """h1 = x @ W1 (bf16 matmul) + attention folds. Output rows
    T1 [NPAD, 384] bf16 (768B rows): cols 0:256 h1 feats, 256:272 a_s
    (bitcast f32), 272:288 a_d (bitcast f32).
    Host extracts the per-dst a_d table. xT input is host-padded to
    NPAD columns (zeros past NPC)."""
    NPAD = dims.NPAD
    NWIN = dims.NWIN
    nc = bacc.Bacc(None, target_bir_lowering=False, num_swdge_queues=NQ)
    with tile.TileContext(nc) as tc:
        with tc.tile_pool(name="dram", bufs=1, space="DRAM") as dram:
            xT = dram.tile([P, NPAD], BF16, kind="ExternalInput")
            W1 = dram.tile([P, 256], F32, kind="ExternalInput")
            att1 = dram.tile([1, 512], F32, kind="ExternalInput")
            T1 = dram.tile([NPAD, 384], BF16, kind="ExternalOutput")
            names = dict(xT=xT.name, W1=W1.name, att1=att1.name, T1=T1.name)
            with tc.tile_pool(name="cst", bufs=1) as cst, \
                 tc.tile_pool(name="wk", bufs=3) as wk, \
                 tc.tile_pool(name="ps", bufs=4, space="PSUM") as ps:
                xTs = cst.tile([P, NPAD], BF16)
                nc.sync.dma_start(xTs[:], xT[:])
                W1s = cst.tile([P, 256], F32)
                nc.sync.dma_start(W1s[:], W1[:])
                att_s = cst.tile([1, 512], F32)
                nc.sync.dma_start(att_s[:], att1[:])
                attb = cst.tile([P, 512], F32)
                nc.gpsimd.partition_broadcast(attb[:, 0:256], att_s[0:1, 0:256])
                nc.gpsimd.partition_broadcast(attb[:, 256:512], att_s[0:1, 256:512])
                tmp = cst.tile([P, 512], F32)
                nc.vector.tensor_tensor(out=tmp[:, 0:256], in0=W1s[:],
                                        in1=attb[:, 0:256], op=ALU.mult)
                nc.vector.tensor_tensor(out=tmp[:, 256:512], in0=W1s[:],
                                        in1=attb[:, 256:512], op=ALU.mult)
                tv = tmp[:].rearrange("p (v h f) -> p v h f", v=2, h=H)
                folds = cst.tile([P, 16], F32)
                nc.vector.tensor_reduce(out=folds[:, 0:8], in_=tv[:, 0],
                                        axis=AX.X, op=ALU.add)
                nc.vector.tensor_reduce(out=folds[:, 8:16], in_=tv[:, 1],
                                        axis=AX.X, op=ALU.add)
                rhs = cst.tile([P, 272], BF16)
                nc.vector.tensor_copy(rhs[:, 0:256], W1s[:])
                nc.vector.tensor_copy(rhs[:, 256:272], folds[:])
                grp = 0
                while grp * GSW < NWIN:
                    ws = list(range(grp * GSW, min((grp + 1) * GSW, NWIN)))
                    nw = len(ws)
                    t14 = wk.tile([P, nw, 288], BF16, tag="t14")
                    for j, w in enumerate(ws):
                        po = ps.tile([P, 272], F32, tag="po")
                        nc.tensor.matmul(out=po[:],
                                         lhsT=xTs[:, w * P:(w + 1) * P],
                                         rhs=rhs[:], start=True, stop=True)
                        nc.scalar.copy(t14[:, j, 0:256], po[:, 0:256])
                        nc.vector.tensor_copy(
                            t14[:, j, 256:288].bitcast(F32)[:, 0:16],
                            po[:, 256:272])
                    nc.sync.dma_start(
                        T1[ws[0] * P:(ws[-1] + 1) * P, 0:288]
                        .rearrange("(w p) c -> p w c", p=P), t14[:])
                    grp += 1
    nc.compile()
    return nc, names


def build_edge(layer, plan, dims: Dims):
    """Edge phase for layer 1 or 2.

    layer 1: gathers T1 rows (576B: h1 256 bf16 + a_s/a_d bitcast f32),
      aggregates per-head h1*alpha (264-col one-hot matmul), outputs
      T2 [NPAD, 256] bf16 rows: 0:128 h2 = relu(out1+b1)@W2 per head,
      128:136 a_s2 bf16, 136:144 a_d2 bf16.
    layer 2: gathers T2 rows (512B), aggregates per-head h2*alpha
      (136-col matmul), outputs OUT [NPAD, 16] f32.

    Per-edge a_d: 1-row matmul broadcasts the transposed dst-slot
    stream; is_equal vs the partition index builds the transposed
    one-hot (fp8); 8-wide matmuls with the fp8 a_d table select a_d."""
    N, BUCK, NWIN, NPAD = dims.N, dims.BUCK, dims.NWIN, dims.NPAD
    supers, windows = plan['supers'], plan['windows']
    gcols, slots = plan['gcols'], plan['slots']
    GELEM = 384 if layer == 1 else 256   # gathered elements per row
    GSTEP = 384 if layer == 1 else 256   # table row stride (elements)
    FW = 256 if layer == 1 else 128      # feature width
    AGG = FW + 8
    OW = 144 if layer == 1 else 16       # output row width
    nc = bacc.Bacc(None, target_bir_lowering=False, num_swdge_queues=NQ)
    qctr = [0]

    def nextq():
        q = qctr[0] % NQ
        qctr[0] += 1
        return q

    with tile.TileContext(nc) as tc:
        with tc.tile_pool(name="dram", bufs=1, space="DRAM") as dram:
            names = {}
            Gt = dram.tile([N, GSTEP], BF16, kind="ExternalInput")
            ADt = dram.tile([NPAD, 8], F32, kind="ExternalInput")
            nb = 32 if layer == 1 else 16
            bias = dram.tile([1, nb], F32, kind="ExternalInput")
            gidx = dram.tile([P, gcols], I16, kind="ExternalInput")
            dstloc = dram.tile([P, slots], BF16, kind="ExternalInput")
            dstlocT = dram.tile([1, slots * P], BF16, kind="ExternalInput")
            names.update(G=Gt.name, AD=ADt.name, bias=bias.name,
                         gidx=gidx.name, dstloc=dstloc.name,
                         dstlocT=dstlocT.name)
            if layer == 1:
                W2 = dram.tile([32, 128], F32, kind="ExternalInput")
                att2 = dram.tile([1, 256], F32, kind="ExternalInput")
                T2o = dram.tile([NPAD, 256], BF16, kind="ExternalOutput")
                names.update(W2=W2.name, att2=att2.name, T2=T2o.name)
            else:
                OUT = dram.tile([NPAD, 16], F32, kind="ExternalOutput")
                names.update(OUT=OUT.name)

            with tc.tile_pool(name="cst", bufs=1) as cst, \
                 tc.tile_pool(name="gp", bufs=2) as gp, \
                 tc.tile_pool(name="gi", bufs=2) as gip, \
                 tc.tile_pool(name="gd", bufs=2) as gdp, \
                 tc.tile_pool(name="wk1", bufs=8) as wk1, \
                 tc.tile_pool(name="wk2", bufs=4) as wk2, \
                 tc.tile_pool(name="wk3", bufs=3) as wk3, \
                 tc.tile_pool(name="wo", bufs=2) as wo, \
                 tc.tile_pool(name="psa", bufs=2, space="PSUM") as psa, \
                 tc.tile_pool(name="psb", bufs=2, space="PSUM") as psb, \
                 tc.tile_pool(name="pso", bufs=1, space="PSUM") as pso, \
                 tc.tile_pool(name="psd", bufs=2, space="PSUM") as psd:
                dstloc_s = cst.tile([P, slots], BF16)
                nc.sync.dma_start(dstloc_s[:], dstloc[:])
                ad_s = cst.tile([P, NWIN, 8], F32)
                nc.sync.dma_start(
                    ad_s[:], ADt[:].rearrange("(w p) c -> p w c", p=P))
                ad8 = cst.tile([P, NWIN, 8], FP8)
                nc.vector.tensor_copy(ad8[:], ad_s[:])
                iota_i = cst.tile([P, P], mybir.dt.int32)
                nc.gpsimd.iota(iota_i[:], pattern=[[1, P]], base=0,
                               channel_multiplier=0)
                iota_bf = cst.tile([P, P], BF16)
                nc.vector.tensor_copy(iota_bf[:], iota_i[:])
                iota_pi = cst.tile([P, 1], mybir.dt.int32)
                nc.gpsimd.iota(iota_pi[:], pattern=[[0, 1]], base=0,
                               channel_multiplier=1)
                iota_pb = cst.tile([P, 1], BF16)
                nc.vector.tensor_copy(iota_pb[:], iota_pi[:])
                ones1 = cst.tile([1, P], BF16)
                nc.vector.memset(ones1[:], 1.0)
                bias_s = cst.tile([1, nb], F32)
                nc.sync.dma_start(bias_s[:], bias[:])
                bias_b = cst.tile([P, nb], F32)
                nc.gpsimd.partition_broadcast(bias_b[:], bias_s[0:1, :])
                if layer == 1:
                    W2s = cst.tile([32, 128], F32)
                    nc.sync.dma_start(W2s[:], W2[:])
                    att2_s = cst.tile([1, 256], F32)
                    nc.sync.dma_start(att2_s[:], att2[:])
                    att2b = cst.tile([32, 256], F32)
                    nc.gpsimd.partition_broadcast(att2b[:, 0:128],
                                                  att2_s[0:1, 0:128])
                    nc.gpsimd.partition_broadcast(att2b[:, 128:256],
                                                  att2_s[0:1, 128:256])
                    tmp2 = cst.tile([32, 256], F32)
                    nc.vector.tensor_tensor(out=tmp2[:, 0:128], in0=W2s[:],
                                            in1=att2b[:, 0:128], op=ALU.mult)
                    nc.vector.tensor_tensor(out=tmp2[:, 128:256], in0=W2s[:],
                                            in1=att2b[:, 128:256], op=ALU.mult)
                    t2v = tmp2[:].rearrange("p (v h f) -> p v h f", v=2, h=H)
                    W2cat = cst.tile([32, 144], F32)
                    nc.vector.tensor_copy(W2cat[:, 0:128], W2s[:])
                    nc.vector.tensor_reduce(out=W2cat[:, 128:136],
                                            in_=t2v[:, 0], axis=AX.X,
                                            op=ALU.add)
                    nc.vector.tensor_reduce(out=W2cat[:, 136:144],
                                            in_=t2v[:, 1], axis=AX.X,
                                            op=ALU.add)
                    ident = cst.tile([P, P], F32)
                    make_identity(nc, ident[:])

                for sup in supers:
                    S = sup['S']
                    ncols = sum(n for (_, n, _, _) in sup['calls']) // 16
                    gc_base = sup['calls'][0][2]
                    gidx_t = gip.tile([P, ncols], I16, tag="gi")
                    nc.sync.dma_start(gidx_t[:],
                                      gidx[:, gc_base:gc_base + ncols])
                    dlT_t = gdp.tile([1, S * P], BF16, tag="dlt")
                    nc.sync.dma_start(
                        dlT_t[:],
                        dstlocT[0:1, sup['slot0'] * P:(sup['slot0'] + S) * P])
                    g_t = gp.tile([P, S, GELEM], BF16, tag="g")
                    for (b, n, gc0, scol) in sup['calls']:
                        nc.gpsimd.dma_gather(
                            g_t[:, scol:scol + n // P, :],
                            Gt[b * BUCK:(b + 1) * BUCK, 0:GELEM],
                            gidx_t[:, gc0 - gc_base:gc0 - gc_base + n // 16],
                            n, n, GELEM, elem_step=GSTEP, queue_num=nextq())
                    ws = sup['windows']
                    nw = len(ws)
                    out_t = wo.tile([P, nw, OW], BF16 if layer == 1 else F32,
                                    tag="out")
                    built = {}

                    def loop1(w):
                        segs = windows[w]['segs']
                        tiles = []
                        for (scol, ns) in segs:
                            p_t = wk1.tile([P, ns, P], BF16, tag="pt")
                            dl_b = dstloc_s[:, sup['slot0'] + scol:
                                            sup['slot0'] + scol + ns] \
                                .unsqueeze(2).to_broadcast([P, ns, P])
                            io_b = iota_bf[:].unsqueeze(1) \
                                .to_broadcast([P, ns, P])
                            nc.vector.tensor_tensor(out=p_t[:], in0=dl_b,
                                                    in1=io_b, op=ALU.is_equal)
                            pts = wk2.tile([P, ns * P], FP8, tag="pts")
                            for c0 in range(0, ns, 4):
                                cn = min(4, ns - c0)
                                pb = psb.tile([P, cn * P], F32, tag="pb")
                                nc.tensor.matmul(
                                    out=pb[:], lhsT=ones1[:],
                                    rhs=dlT_t[0:1, (scol + c0) * P:
                                              (scol + c0 + cn) * P],
                                    start=True, stop=True)
                                nc.vector.tensor_tensor(
                                    out=pts[:, c0 * P:(c0 + cn) * P],
                                    in0=pb[:],
                                    in1=iota_pb[:].to_broadcast([P, cn * P]),
                                    op=ALU.is_equal)
                            adE = psd.tile([P, ns * 8], F32, tag="adE")
                            for k in range(ns):
                                nc.tensor.matmul(
                                    out=adE[:, k * 8:(k + 1) * 8],
                                    lhsT=pts[:, k * P:(k + 1) * P],
                                    rhs=ad8[:, w, :], start=True, stop=True)
                            if layer == 1:
                                a_s_ap = g_t[:] \
                                    .rearrange("p s e -> p (s e)") \
                                    .bitcast(F32) \
                                    .rearrange("p (s e) -> p s e", e=192) \
                                    [:, scol:scol + ns, 128:136]
                            else:
                                a_s_ap = g_t[:, scol:scol + ns, 128:136]
                            et = wk2.tile([P, ns, 8], F32, tag="et")
                            nc.vector.tensor_tensor(
                                out=et[:], in0=a_s_ap,
                                in1=adE[:].rearrange("p (s e) -> p s e", e=8),
                                op=ALU.add)
                            nc.vector.scalar_tensor_tensor(
                                out=et[:], in0=et[:], scalar=0.2, in1=et[:],
                                op0=ALU.mult, op1=ALU.max)
                            rhs_t = wk1.tile([P, ns, AGG], BF16, tag="rhs")
                            nc.scalar.activation(rhs_t[:, :, FW:FW + 8],
                                                 et[:], AF.Exp)
                            wexp_b = rhs_t[:, :, FW:FW + 8].unsqueeze(3) \
                                .to_broadcast([P, ns, 8, FW // 8])
                            g_v = g_t[:, scol:scol + ns, 0:FW] \
                                .rearrange("p s (h f) -> p s h f", h=H)
                            nc.vector.tensor_tensor(
                                out=rhs_t[:, :, 0:FW]
                                .rearrange("p s (h f) -> p s h f", h=H),
                                in0=g_v, in1=wexp_b, op=ALU.mult)
                            tiles.append((p_t, rhs_t, ns))
                        built[w] = tiles

                    def loop2(w, wi):
                        tiles = built.pop(w)
                        nslot = sum(ns for (_, _, ns) in tiles)
                        agg = psa.tile([P, AGG], F32, tag="agg")
                        sdone = 0
                        for (p_t, rhs_t, ns) in tiles:
                            for k in range(ns):
                                nc.tensor.matmul(
                                    out=agg[:], lhsT=p_t[:, k, :],
                                    rhs=rhs_t[:, k, :],
                                    start=(sdone + k == 0),
                                    stop=(sdone + k == nslot - 1))
                            sdone += ns
                        zr = wk3.tile([P, 8], F32, tag="zr")
                        nc.vector.tensor_scalar_add(zr[:], agg[:, FW:FW + 8],
                                                    1e-16)
                        nc.vector.reciprocal(zr[:], zr[:])
                        nc.vector.tensor_scalar_mul(zr[:], zr[:], 1.0 / H)
                        zrb = zr[:].unsqueeze(2).to_broadcast([P, H, FW // 8])
                        hn = wk3.tile([P, FW], F32, tag="hn")
                        nc.vector.tensor_tensor(
                            out=hn[:].rearrange("p (h f) -> p h f", h=H),
                            in0=agg[:, 0:FW].rearrange("p (h f) -> p h f", h=H),
                            in1=zrb, op=ALU.mult)
                        if layer == 1:
                            o1 = wk3.tile([P, 32], F32, tag="o1")
                            nc.vector.tensor_reduce(
                                out=o1[:],
                                in_=hn[:].rearrange("p (h f) -> p f h", h=H),
                                axis=AX.X, op=ALU.add)
                            nc.vector.tensor_tensor(out=o1[:], in0=o1[:],
                                                    in1=bias_b[:, 0:32],
                                                    op=ALU.add)
                            nc.vector.tensor_scalar_max(o1[:], o1[:], 0.0)
                            hT = pso.tile([32, P], F32, tag="hT")
                            nc.tensor.transpose(hT[:], o1[:], ident[:])
                            hTs = wk3.tile([32, P], F32, tag="hTs")
                            nc.vector.tensor_copy(hTs[:], hT[:])
                            h2a = pso.tile([P, 144], F32, tag="h2a")
                            nc.tensor.matmul(out=h2a[:], lhsT=hTs[:],
                                             rhs=W2cat[:], start=True,
                                             stop=True)
                            nc.scalar.copy(out_t[:, wi, :], h2a[:])
                        else:
                            nc.vector.tensor_reduce(
                                out=out_t[:, wi, :],
                                in_=hn[:].rearrange("p (h f) -> p f h", h=H),
                                axis=AX.X, op=ALU.add)
                            nc.vector.tensor_tensor(out=out_t[:, wi, :],
                                                    in0=out_t[:, wi, :],
                                                    in1=bias_b[:, 0:16],
                                                    op=ALU.add)

                    prev = []
                    for w in ws:
                        loop1(w)
                        prev.append(w)
                        if len(prev) > 1:
                            loop2(prev[0], ws.index(prev[0]))
                            prev.pop(0)
                    for w in prev:
                        loop2(w, ws.index(w))
                    dst_ap = (T2o if layer == 1 else OUT)
                    nc.sync.dma_start(
                        dst_ap[ws[0] * P:(ws[-1] + 1) * P, 0:OW]
                        .rearrange("(w p) c -> p w c", p=P), out_t[:])
    nc.compile()
    return nc, names


# ---------------- driver ----------------


def _run_pipeline(inputs, dims, trace=False):
    x = np.asarray(inputs['x'], np.float32)
    ei = np.asarray(inputs['edge_index'])
    W1 = np.ascontiguousarray(np.asarray(inputs['W1'], np.float32))
    as1 = np.asarray(inputs['att_src1'], np.float32)
    ad1 = np.asarray(inputs['att_dst1'], np.float32)
    b1 = np.asarray(inputs['b1'], np.float32)
    W2 = np.ascontiguousarray(np.asarray(inputs['W2'], np.float32))
    as2 = np.asarray(inputs['att_src2'], np.float32)
    ad2 = np.asarray(inputs['att_dst2'], np.float32)
    b2 = np.asarray(inputs['b2'], np.float32)
    NC, NPC, NPAD = dims.NCORES, dims.NPC, dims.NPAD

    plan, streams = build_plans(ei, dims)
    times = {}

    nc1, n1 = build_dense1(dims)
    att1 = np.ascontiguousarray(np.concatenate(
        [as1.reshape(-1), ad1.reshape(-1)]).reshape(1, -1).astype(np.float32))
    ins1 = []
    for c in range(NC):
        xTp = np.zeros((P, NPAD), dtype=ml_dtypes.bfloat16)
        xTp[:, :NPC] = x[c * NPC:(c + 1) * NPC, :].T.astype(ml_dtypes.bfloat16)
        ins1.append({n1['xT']: xTp, n1['W1']: W1, n1['att1']: att1})
    r1 = bass_utils.run_bass_kernel_spmd(nc1, ins1, core_ids=list(range(NC)),
                                         trace=trace)
    times['dense1'] = r1.exec_time_ns
    t1_shards = [r1.results[c][n1['T1']] for c in range(NC)]
    T1full = np.ascontiguousarray(
        np.concatenate([t[:NPC] for t in t1_shards]))
    ad1_shards = []
    for c in range(NC):
        a = np.zeros((NPAD, 8), np.float32)
        a[:NPC] = np.ascontiguousarray(
            t1_shards[c][:NPC, 272:288]).view(np.float32)
        ad1_shards.append(a)

    nc2, n2 = build_edge(1, plan, dims)
    att2 = np.ascontiguousarray(np.concatenate(
        [as2.reshape(-1), ad2.reshape(-1)]).reshape(1, -1).astype(np.float32))
    ins2 = [{n2['G']: T1full, n2['AD']: ad1_shards[c], n2['W2']: W2,
             n2['att2']: att2,
             n2['bias']: np.ascontiguousarray(b1.reshape(1, -1)),
             n2['gidx']: streams[c]['gidx'],
             n2['dstloc']: streams[c]['dstloc'],
             n2['dstlocT']: streams[c]['dstlocT']} for c in range(NC)]
    r2 = bass_utils.run_bass_kernel_spmd(nc2, ins2, core_ids=list(range(NC)),
                                         trace=trace)
    times['edge1'] = r2.exec_time_ns
    t2_shards = [r2.results[c][n2['T2']] for c in range(NC)]
    T2full = np.ascontiguousarray(
        np.concatenate([t[:NPC] for t in t2_shards]))
    ad2_shards = []
    for c in range(NC):
        a = np.zeros((NPAD, 8), np.float32)
        a[:NPC] = t2_shards[c][:NPC, 136:144].astype(np.float32)
        ad2_shards.append(a)

    nc3, n3 = build_edge(2, plan, dims)
    ins3 = [{n3['G']: T2full, n3['AD']: ad2_shards[c],
             n3['bias']: np.ascontiguousarray(b2.reshape(1, -1)),
             n3['gidx']: streams[c]['gidx'],
             n3['dstloc']: streams[c]['dstloc'],
             n3['dstlocT']: streams[c]['dstlocT']} for c in range(NC)]
    r3 = bass_utils.run_bass_kernel_spmd(nc3, ins3, core_ids=list(range(NC)),
                                         trace=trace)
    times['edge2'] = r3.exec_time_ns
    out = np.concatenate([r3.results[c][n3['OUT']][:NPC] for c in range(NC)])
    return np.ascontiguousarray(out.astype(np.float32)), times


def kernel(**inputs):
    out, _ = _run_pipeline(inputs, Dims(), trace=False)
    return out
